# revision 59
# baseline (speedup 1.0000x reference)
"""GAT 2-layer message-passing network on 8 TRN2 NeuronCores (Bass/Tile).

v3: dispatch/instruction-count optimized.

Strategy (dst-sharded):
 - Host: add self loops, sort edges by dst, shard dst-node ranges across cores.
   Each core owns nodes [c*NPC, (c+1)*NPC) and ALL edges into them.
 - Edge slots: per dst-block of 128 nodes, edges sub-grouped by src chunk
   (4 chunks of CH rows so int16 indices work), each (block,chunk) run padded
   to x128 slots = tiles. Superblocks of SBG blocks share gather calls.
 - Phase A (replicated): full feature table htab[n] = [h|a_src|pad] bf16
   [Np, 384] (768B rows for dma_gather), + local stats table sloc
   [NPCp, H] bf16 (a_dst of the core's own nodes), batched 8 tiles per DMA.
 - Phase B (L1): per sb: dma_gather htab rows by src (4 chunk calls);
   oT one-hot ([dst, slot]) via is_equal against the host-replicated
   dlocR table (no PE broadcast); per-tile matmuls oT x adw -> per-edge
   a_dst; ex = exp(lrelu(asrc+adst)) batched per sb; msg in-place in the
   gather buffer; one-hot oh from dloc2d vs iota; per-block PSUM matmul
   accumulation; per-sb batched normalize + b1 + relu; h2aug = relu @ W2aug
   via PE transpose; batched h2loc (AG input) + h2pad stores.
 - AllGather h2loc -> h2tab [N,4] f32; repack into h2tabp [Npp, 64] f32 rows.
 - Phase C (L2): same slots: gather h2tabp by src; same oT/oh structure;
   4-wide bf16 messages; one-hot matmuls; per-sb batched normalize, +b2,
   log_softmax -> out [NPC, 2] f32.
"""
import sys

if "/opt/trn_rl_repo" not in sys.path:
    sys.path.insert(0, "/opt/trn_rl_repo")

import math
import numpy as np
import ml_dtypes

import concourse.bass as bass
import concourse.bacc as bacc
import concourse.mybir as mybir
import concourse.tile as tile
from concourse import bass_utils

P = 128
NEG = 0.2
NCHUNK = 4
NQUEUE = 4
BAT = 8                      # phase-A tiles per DMA batch

# Tile's DMA sem-lane assignment round-robins over all DMAs of a DGE class,
# which breaks the per-lane FIFO assumption when DMAs run on multiple HW
# rings (out-of-order completion across rings under one counting sem):
#  - SWDGE (Pool) on multiple queues -> lane == queue_num.
#  - HWDGE from both SP (sync) and ACT (scalar) rings -> SP lanes 0-3,
#    ACT lanes 4-7 (per-engine round-robin).
from concourse import tile_sem_assignment as _tsa  # noqa: E402

if not getattr(_tsa.TileClockTick, "_qaware_patched", False):
    _orig_assign_tick = _tsa.TileClockTick._assign_tick

    def _qaware_assign_tick(self, inst):
        if isinstance(inst, _tsa.DMAInst):
            if inst.engine == mybir.EngineType.Pool:
                self.next_sw_dma_idx = getattr(inst, "queue_num", 0) or 0
            elif inst.engine in (mybir.EngineType.SP,
                                 mybir.EngineType.Activation):
                if not hasattr(self, "_hw_rr"):
                    self._hw_rr = [0, 0]
                w = 1 if inst.engine == mybir.EngineType.Activation else 0
                self.next_hw_dma_idx = w * 4 + (self._hw_rr[w] % 4)
                self._hw_rr[w] += 1
        return _orig_assign_tick(self, inst)

    _tsa.TileClockTick._assign_tick = _qaware_assign_tick
    _tsa.TileClockTick._qaware_patched = True


def _wrap16(flat):
    """[n] -> [128, n//16] wrapped in 16 partitions, replicated x8."""
    w = flat.reshape(-1, 16).T
    return np.tile(w, (8, 1))


# ----------------------------------------------------------------------------
# host-side data prep
# ----------------------------------------------------------------------------

def prep(inputs, cfg):
    N, F, H, C, CLS, NC = cfg["N"], cfg["F"], cfg["H"], cfg["C"], cfg["CLS"], cfg["NC"]
    SBG = cfg.get("SBG", 4)
    x = np.asarray(inputs["x"], np.float32)
    ei = np.asarray(inputs["edge_index"])
    W1 = np.asarray(inputs["W1"], np.float32)
    as1 = np.asarray(inputs["att_src1"], np.float32)
    ad1 = np.asarray(inputs["att_dst1"], np.float32)
    b1 = np.asarray(inputs["b1"], np.float32)
    W2 = np.asarray(inputs["W2"], np.float32)
    as2 = np.asarray(inputs["att_src2"], np.float32)
    ad2 = np.asarray(inputs["att_dst2"], np.float32)
    b2 = np.asarray(inputs["b2"], np.float32)

    HC = H * C
    R1 = HC + 2 * H                      # live row payload [h | asrc | adst]
    RG = 128 * math.ceil(R1 / 128)       # htab gather row elems (bf16, 256B mult)
    NPC = N // NC
    NB = math.ceil(NPC / P)
    NPCp = NB * P                        # padded local rows
    NT = (N + P - 1) // P
    Np = NT * P
    # chunk base, tile-aligned so phase-A stores land in per-chunk ranges
    # (lets chunk-q gathers start before the whole table is written)
    CHB = math.ceil(N / NCHUNK / P) * P
    assert Np - (NCHUNK - 1) * CHB < 32768  # last-chunk slice
    assert CHB < 32768 and NPCp < 32768

    # ---- weights / constants -------------------------------------------------
    W1r = W1.reshape(F, H, C)
    Wsrc = np.einsum("fhc,hc->fh", W1r, as1)
    Wdst = np.einsum("fhc,hc->fh", W1r, ad1)
    W1aug = np.concatenate([W1, Wsrc, Wdst], axis=1)          # [F, R1]
    Wsrc2 = W2 @ as2.reshape(CLS, 1)
    Wdst2 = W2 @ ad2.reshape(CLS, 1)
    W2aug = np.concatenate([W2, Wsrc2, Wdst2], axis=1)        # [HC, 4]

    bf16 = ml_dtypes.bfloat16
    xT = np.zeros((F, Np), dtype=bf16)
    xT[:, :N] = x.T.astype(bf16)
    W1aug_b = W1aug.astype(bf16)
    W2aug_b = W2aug.astype(bf16)
    b1rep = np.tile(b1[None, :], (P, 1)).astype(bf16)
    b2rep = np.tile(b2[None, :], (P, 1)).astype(np.float32)
    iota = np.tile(np.arange(P, dtype=np.float32)[None, :], (P, 1)).astype(bf16)
    ident = np.eye(P, dtype=bf16)

    # ---- edges ---------------------------------------------------------------
    src_all = np.concatenate([ei[0], np.arange(N, dtype=ei.dtype)]).astype(np.int64)
    dst_all = np.concatenate([ei[1], np.arange(N, dtype=ei.dtype)]).astype(np.int64)
    order = np.argsort(dst_all, kind="stable")
    src_s = src_all[order]
    dst_s = dst_all[order]
    chunk_s = src_s // CHB

    cnts = np.zeros((NC, NB, NCHUNK), np.int64)
    for c in range(NC):
        for b in range(NB):
            base = c * NPC + b * P
            hi = min(base + P, (c + 1) * NPC)
            lo_i = np.searchsorted(dst_s, base)
            hi_i = np.searchsorted(dst_s, hi)
            ch = chunk_s[lo_i:hi_i]
            for q in range(NCHUNK):
                cnts[c, b, q] = (ch == q).sum()
    Trun = np.ceil(cnts / P).astype(np.int64).max(axis=0)     # [NB, NCHUNK]

    # superblocks
    sblocks = [list(range(i, min(i + SBG, NB))) for i in range(0, NB, SBG)]
    # slot layout: per sb: for q: for b in sb: Trun[b,q] tiles
    sb_meta = []
    tile_base = 0
    for blist in sblocks:
        segs = []           # per q: (seg_tile_base_global, segT)
        runs = {b: [] for b in blist}   # block -> [(tile_global, T)]
        sb_base = tile_base
        for q in range(NCHUNK):
            segT = int(Trun[blist, q].sum())
            segs.append((tile_base, segT))
            tb = tile_base
            for b in blist:
                t = int(Trun[b, q])
                if t:
                    runs[b].append((tb, t))
                tb += t
            tile_base += segT
        sb_meta.append(dict(base=sb_base, S=tile_base - sb_base, segs=segs,
                            blocks=blist, runs=runs))
    Tsum = tile_base

    # per-core slot-value arrays
    ihsrc_w = np.zeros((NC, P, Tsum * 8), np.int16)
    dloc2d = np.full((NC, P, Tsum), 255.0, bf16)
    dlocR_a = np.zeros((NC, P, Tsum * P), bf16)
    for c in range(NC):
        ihsrc = np.zeros(Tsum * P, np.int16)
        dloc = np.full(Tsum * P, 255.0, np.float32)
        core_lo = np.searchsorted(dst_s, c * NPC)
        core_hi = np.searchsorted(dst_s, (c + 1) * NPC)
        cs, cd, cq = (src_s[core_lo:core_hi], dst_s[core_lo:core_hi],
                      chunk_s[core_lo:core_hi])
        # edges sorted by (dst, chunk); regroup per (block, chunk)
        for sb in sb_meta:
            for q in range(NCHUNK):
                for b in sb["blocks"]:
                    t = int(Trun[b, q])
                    if t == 0:
                        continue
                    # this block+chunk's edges (mask within the dst range)
                    base = c * NPC + b * P
                    hi = min(base + P, (c + 1) * NPC)
                    seg = slice(np.searchsorted(cd, base), np.searchsorted(cd, hi))
                    m = cq[seg] == q
                    es, ed = cs[seg][m], cd[seg][m]
                    n = len(es)
                    assert n <= t * P
                    # locate this run's global tile index (runs are in q order)
                    tg = None
                    for (tgi, tti) in sb["runs"][b]:
                        s0, sT = sb["segs"][q]
                        if s0 <= tgi < s0 + sT:
                            tg = tgi
                            break
                    assert tg is not None
                    s0 = tg * P
                    ihsrc[s0:s0 + n] = (es - q * CHB).astype(np.int16)
                    dloc[s0:s0 + n] = (ed - (c * NPC + b * P)).astype(np.float32)
        ihsrc_w[c] = _wrap16(ihsrc)
        dloc2d[c] = dloc.reshape(Tsum, P).T.astype(bf16)
        # dloc[s] - p per partition: oT = (dlz == 0) via fast tensor_scalar
        dlocR_a[c] = (dloc[None, :]
                      - np.arange(P, dtype=np.float32)[:, None]).astype(bf16)

    shared = {
        "xT": xT, "W1aug": W1aug_b, "W2aug": W2aug_b, "b1rep": b1rep,
        "b2rep": b2rep, "iota": iota, "ident": ident,
    }
    in_maps = []
    for c in range(NC):
        m = dict(shared)
        xl = np.zeros((F, NPCp), dtype=bf16)
        xl[:, :NPC] = xT[:, c * NPC:c * NPC + NPC]
        m["xTloc"] = xl
        m["ihsrc"] = ihsrc_w[c]
        m["dloc2d"] = dloc2d[c]
        m["dlocR"] = dlocR_a[c]
        in_maps.append(m)

    meta = dict(cfg, R1=R1, RG=RG, HC=HC, NPC=NPC, NPCp=NPCp, NB=NB, NT=NT,
                Np=Np, CHB=CHB, Tsum=Tsum, sb_meta=sb_meta, SBG=SBG)
    return in_maps, meta


# ----------------------------------------------------------------------------
# device program
# ----------------------------------------------------------------------------

def _sub(ap, elem_off, dims):
    return bass.AP(ap.tensor, ap.offset + elem_off, [ap.ap[0], *list(dims)])


def build(meta, nc=None):
    N, F, H, C, CLS = meta["N"], meta["F"], meta["H"], meta["C"], meta["CLS"]
    NC, R1, RG, HC = meta["NC"], meta["R1"], meta["RG"], meta["HC"]
    NPC, NPCp, NB, NT, Np = (meta["NPC"], meta["NPCp"], meta["NB"], meta["NT"],
                             meta["Np"])
    CHB, Tsum = meta["CHB"], meta["Tsum"]
    sb_meta = meta["sb_meta"]
    R2 = CLS + 2
    RB2 = 64                           # f32 row elems for L2 gather tables

    f32, bf16, i16 = mybir.dt.float32, mybir.dt.bfloat16, mybir.dt.int16

    if nc is None:
        nc = bacc.Bacc("TRN2", target_bir_lowering=False, debug=False,
                       num_devices=NC, num_swdge_queues=NQUEUE)

    MAXT = 7                 # tiles per dma_gather call (<=896 descs, carveout 1024)
    qrr = [0]

    def gather_split(out_tile, rel, segT, elem, table, ix_tile):
        """Split a segment gather into <=MAXT-tile calls, round-robin queues."""
        done = 0
        while done < segT:
            tt = min(MAXT, segT - done)
            r = rel + done
            nc.gpsimd.dma_gather(
                bass.AP(out_tile[:].tensor, out_tile[:].offset + r * elem,
                        [out_tile[:].ap[0], [elem, tt], [1, elem]]),
                table,
                ix_tile[:, r * 8:(r + tt) * 8],
                tt * P, tt * P, elem,
                queue_num=qrr[0] % NQUEUE,
            )
            qrr[0] += 1
            done += tt

    xT_d = nc.dram_tensor("xT", [F, Np], bf16, kind="ExternalInput")
    xTl_d = nc.dram_tensor("xTloc", [F, NPCp], bf16, kind="ExternalInput")
    W1aug_d = nc.dram_tensor("W1aug", [F, R1], bf16, kind="ExternalInput")
    W2aug_d = nc.dram_tensor("W2aug", [HC, R2], bf16, kind="ExternalInput")
    b1rep_d = nc.dram_tensor("b1rep", [P, HC], bf16, kind="ExternalInput")
    b2rep_d = nc.dram_tensor("b2rep", [P, CLS], f32, kind="ExternalInput")
    iota_d = nc.dram_tensor("iota", [P, P], bf16, kind="ExternalInput")
    ident_d = nc.dram_tensor("ident", [P, P], bf16, kind="ExternalInput")
    ihsrc_d = nc.dram_tensor("ihsrc", [P, Tsum * 8], i16, kind="ExternalInput")
    dloc_d = nc.dram_tensor("dloc2d", [P, Tsum], bf16, kind="ExternalInput")
    dlocR_d = nc.dram_tensor("dlocR", [P, Tsum * P], bf16, kind="ExternalInput")
    out_d = nc.dram_tensor("out", [NPC, CLS], f32, kind="ExternalOutput")

    # per-chunk h tables (separate tensors so chunk-q gathers only depend on
    # chunk-q phase-A stores)
    nchrows = [CHB] * (NCHUNK - 1) + [Np - (NCHUNK - 1) * CHB]
    htabq = [nc.dram_tensor(f"htab{q}", [nchrows[q], RG], bf16, kind="Internal")
             for q in range(NCHUNK)]
    sloc = nc.dram_tensor("sloc", [NPCp, H], bf16, kind="Internal")
    h2loc = nc.dram_tensor("h2loc", [NPC, R2], f32, kind="Internal")
    h2pad = nc.dram_tensor("h2pad", [NPCp, R2], f32, kind="Internal")
    h2tab = nc.dram_tensor("h2tab", [N, R2], f32, kind="Internal",
                           addr_space="Shared" if NC > 4 else "Local")
    h2tabp = nc.dram_tensor("h2tabp", [N, RB2], f32, kind="Internal")

    FA = min(P, F)
    FB = F - FA
    NCK = (HC + P - 1) // P

    with tile.TileContext(nc) as tc:
        with tc.tile_pool(name="const", bufs=1) as cp:
            w1a = cp.tile([FA, R1], bf16)
            nc.sync.dma_start(out=w1a[:], in_=W1aug_d[0:FA, :])
            if FB:
                w1b = cp.tile([FB, R1], bf16)
                nc.sync.dma_start(out=w1b[:], in_=W1aug_d[FA:F, :])
            w2s = []
            for k in range(NCK):
                kk = min(P, HC - k * P)
                w2k = cp.tile([kk, R2], bf16, name=f"w2k{k}")
                nc.sync.dma_start(out=w2k[:], in_=W2aug_d[k * P:k * P + kk, :])
                w2s.append(w2k)
            b1s = cp.tile([P, HC], bf16)
            nc.sync.dma_start(out=b1s[:], in_=b1rep_d[:, :])
            b2s = cp.tile([P, CLS], f32)
            nc.sync.dma_start(out=b2s[:], in_=b2rep_d[:, :])
            iot = cp.tile([P, P], bf16)
            nc.sync.dma_start(out=iot[:], in_=iota_d[:, :])
            idn = cp.tile([P, P], bf16)
            nc.sync.dma_start(out=idn[:], in_=ident_d[:, :])
            dlc = cp.tile([P, Tsum], bf16)
            nc.sync.dma_start(out=dlc[:], in_=dloc_d[:, :])

            # ---------------- Phase A: feature tables ------------------------
            # local a_dst stats FIRST so phase-B adw loads unblock early.
            # pa (SBUF) stays open through B/C so phase-B tiles don't reuse
            # its addresses (address-reuse WAR would chain gathers behind
            # the whole of phase A); only the PSUM pool closes.
            import contextlib
            _pa_stack = contextlib.ExitStack()
            pa = _pa_stack.enter_context(tc.tile_pool(name="pa", bufs=2))
            with tc.tile_pool(name="psa", bufs=4, space="PSUM") as psa:
                for g8 in range(0, NPCp // P, BAT):
                    nb8 = min(BAT, NPCp // P - g8)
                    xa = pa.tile([FA, BAT * P], bf16, tag="xla")
                    nc.sync.dma_start(out=xa[:, :nb8 * P],
                                      in_=xTl_d[0:FA, g8 * P:(g8 + nb8) * P])
                    if FB:
                        xb = pa.tile([FB, BAT * P], bf16, tag="xlb")
                        nc.sync.dma_start(out=xb[:, :nb8 * P],
                                          in_=xTl_d[FA:F, g8 * P:(g8 + nb8) * P])
                    ss8 = pa.tile([P, BAT * H], bf16, tag="ss8")
                    for k in range(nb8):
                        ps = psa.tile([P, H], f32, tag="pss")
                        nc.tensor.matmul(out=ps[:], lhsT=xa[:, k * P:(k + 1) * P],
                                         rhs=w1a[:, HC + H:HC + 2 * H],
                                         start=True, stop=(FB == 0))
                        if FB:
                            nc.tensor.matmul(out=ps[:],
                                             lhsT=xb[:, k * P:(k + 1) * P],
                                             rhs=w1b[:, HC + H:HC + 2 * H],
                                             start=False, stop=True)
                        nc.vector.tensor_copy(out=ss8[:, k * H:(k + 1) * H],
                                              in_=ps[:])
                    nc.sync.dma_start(
                        out=bass.AP(sloc, g8 * P * H,
                                    [[H, P], [P * H, nb8], [1, H]]),
                        in_=_sub(ss8[:], 0, [[H, nb8], [1, H]]))
                for q in range(NCHUNK):
                    qt0 = q * CHB // P
                    qnt = min(NT, (q * CHB + nchrows[q]) // P) - qt0
                    for g8 in range(qt0, qt0 + qnt, BAT):
                        nb8 = min(BAT, qt0 + qnt - g8)
                        xa = pa.tile([FA, BAT * P], bf16, tag="xa")
                        nc.sync.dma_start(out=xa[:, :nb8 * P],
                                          in_=xT_d[0:FA, g8 * P:(g8 + nb8) * P])
                        if FB:
                            xb = pa.tile([FB, BAT * P], bf16, tag="xb")
                            nc.sync.dma_start(
                                out=xb[:, :nb8 * P],
                                in_=xT_d[FA:F, g8 * P:(g8 + nb8) * P])
                        hs8 = pa.tile([P, BAT * R1], bf16, tag="hs8")
                        for k in range(nb8):
                            ph = psa.tile([P, R1], f32, tag="ph")
                            nc.tensor.matmul(out=ph[:],
                                             lhsT=xa[:, k * P:(k + 1) * P],
                                             rhs=w1a[:], start=True,
                                             stop=(FB == 0))
                            if FB:
                                nc.tensor.matmul(out=ph[:],
                                                 lhsT=xb[:, k * P:(k + 1) * P],
                                                 rhs=w1b[:], start=False,
                                                 stop=True)
                            nc.vector.tensor_copy(
                                out=hs8[:, k * R1:(k + 1) * R1], in_=ph[:])
                        nc.sync.dma_start(
                            out=bass.AP(htabq[q], (g8 - qt0) * P * RG,
                                        [[RG, P], [P * RG, nb8], [1, R1]]),
                            in_=_sub(hs8[:], 0, [[R1, nb8], [1, R1]]))

            # ---------------- Phase B: L1 edge pass --------------------------
            with tc.tile_pool(name="pbg", bufs=2) as pbg, \
                 tc.tile_pool(name="pbb", bufs=2) as pbb, \
                 tc.tile_pool(name="psb", bufs=2, space="PSUM") as psb, \
                 tc.tile_pool(name="pst", bufs=2, space="PSUM") as pst, \
                 tc.tile_pool(name="psh", bufs=2, space="PSUM") as psh, \
                 tc.tile_pool(name="psa2", bufs=2, space="PSUM") as psa2:
                for sb in sb_meta:
                    base, S = sb["base"], sb["S"]
                    nblk = len(sb["blocks"])
                    b0 = sb["blocks"][0]
                    g = pbg.tile([P, S * RG], bf16, tag="g")
                    ixs = pbg.tile([P, S * 8], i16, tag="ixs")
                    nc.scalar.dma_start(out=ixs[:],
                                        in_=ihsrc_d[:, base * 8:(base + S) * 8])
                    for q in range(NCHUNK):
                        tb, segT = sb["segs"][q]
                        if segT == 0:
                            continue
                        gather_split(g, tb - base, segT, RG, htabq[q][:, :], ixs)
                    # a_dst window for the sb's blocks  [P, nblk*H] bf16
                    adw = pbg.tile([P, 8 * H], bf16, tag="adw")
                    nc.scalar.dma_start(
                        out=adw[:, :nblk * H],
                        in_=bass.AP(sloc, b0 * P * H,
                                    [[H, P], [P * H, nblk], [1, H]]))
                    # one-hot for all slots  [P, S*P] bf16 (two halves)
                    oh = pbb.tile([P, S * P], bf16, tag="oh", bufs=1)
                    OH2 = (S + 1) // 2
                    for z0 in range(0, S, OH2):
                        nz = min(OH2, S - z0)
                        nc.vector.tensor_tensor(
                            out=_sub(oh[:], z0 * P, [[P, nz], [1, P]]),
                            in0=_sub(iot[:], 0, [[0, nz], [1, P]]),
                            in1=_sub(dlc[:], base + z0, [[1, nz], [0, P]]),
                            op=mybir.AluOpType.is_equal)
                    # O_T: [d, slot] one-hot via host (dloc - p) table
                    dlR = pbg.tile([P, S * P], bf16, tag="dlR")
                    nc.scalar.dma_start(out=dlR[:],
                                        in_=dlocR_d[:, base * P:(base + S) * P])
                    oTs = pbb.tile([P, S * P], bf16, tag="oTs", bufs=1)
                    nc.vector.tensor_scalar(
                        out=oTs[:], in0=dlR[:], scalar1=0.0, scalar2=None,
                        op0=mybir.AluOpType.is_equal)
                    # per-edge a_dst: oT x adw matmuls -> PSUM [P, S*H]
                    pad = psa2.tile([P, S * H], f32, tag="pad")
                    for bi, b in enumerate(sb["blocks"]):
                        for (tg, tt) in sb["runs"][b]:
                            for t in range(tt):
                                rel = tg - base + t
                                nc.tensor.matmul(
                                    out=pad[:, rel * H:(rel + 1) * H],
                                    lhsT=oTs[:, rel * P:(rel + 1) * P],
                                    rhs=adw[:, bi * H:(bi + 1) * H],
                                    start=True, stop=True,
                                    skip_group_check=True)
                    # ex = exp(lrelu(asrc+adst)) for all slots  [P, S*H] f32
                    ex = pbb.tile([P, S * H], f32, tag="ex", bufs=1)
                    nc.vector.tensor_tensor(
                        out=ex[:].rearrange("p (t h) -> p t h", t=S),
                        in0=_sub(g[:], HC, [[RG, S], [1, H]]),
                        in1=pad[:].rearrange("p (t h) -> p t h", t=S),
                        op=mybir.AluOpType.add)
                    tmp = pbb.tile([P, S * H], f32, tag="tmp", bufs=1)
                    nc.vector.tensor_scalar_mul(out=tmp[:], in0=ex[:], scalar1=NEG)
                    nc.vector.tensor_tensor(out=ex[:], in0=ex[:], in1=tmp[:],
                                            op=mybir.AluOpType.max)
                    nc.scalar.activation(out=ex[:], in_=ex[:],
                                         func=mybir.ActivationFunctionType.Exp)
                    # all-2B operands for the big broadcast multiply
                    exb = pbb.tile([P, S * H], bf16, tag="exb", bufs=1)
                    nc.vector.tensor_copy(out=exb[:], in_=ex[:])
                    # msg in-place: cols 0:HC *= ex ; cols HC:HC+2H = ex
                    # (two halves so per-tile aggs unblock before the whole
                    # multiply finishes)
                    SH2 = (S + 1) // 2
                    for s0 in range(0, S, SH2):
                        ns = min(SH2, S - s0)
                        nc.vector.tensor_tensor(
                            out=_sub(g[:], s0 * RG, [[RG, ns], [C, H], [1, C]]),
                            in0=_sub(g[:], s0 * RG, [[RG, ns], [C, H], [1, C]]),
                            in1=_sub(exb[:], s0 * H, [[H, ns], [1, H], [0, C]]),
                            op=mybir.AluOpType.mult)
                        nc.vector.tensor_copy(
                            out=_sub(g[:], s0 * RG + HC, [[RG, ns], [1, H]]),
                            in_=_sub(exb[:], s0 * H, [[H, ns], [1, H]]))
                        nc.vector.tensor_copy(
                            out=_sub(g[:], s0 * RG + HC + H, [[RG, ns], [1, H]]),
                            in_=_sub(exb[:], s0 * H, [[H, ns], [1, H]]))
                    # per-block accumulation
                    po = pbb.tile([P, 4 * R1], f32, tag="po", bufs=1)
                    for bi, b in enumerate(sb["blocks"]):
                        runs = sb["runs"][b]
                        ntile = sum(t for _, t in runs)
                        pso = psb.tile([P, R1], f32, tag="pso")
                        ti = 0
                        for (tg, tt) in runs:
                            for t in range(tt):
                                rel = tg - base + t
                                nc.tensor.matmul(
                                    out=pso[:],
                                    lhsT=oh[:, rel * P:(rel + 1) * P],
                                    rhs=g[:, rel * RG:rel * RG + R1],
                                    start=(ti == 0), stop=(ti == ntile - 1))
                                ti += 1
                        nc.vector.tensor_copy(out=po[:, bi * R1:(bi + 1) * R1],
                                              in_=pso[:])
                    # batched normalize + bias + relu over the sb's blocks
                    den = pbb.tile([P, 4 * H], f32, tag="den")
                    nc.vector.tensor_scalar_max(
                        out=den[:, :nblk * H],
                        in0=_sub(po[:], HC, [[R1, nblk], [1, H]]),
                        scalar1=1e-20)
                    rde = pbb.tile([P, 4 * H], f32, tag="rde")
                    nc.vector.reciprocal(out=rde[:, :nblk * H],
                                         in_=den[:, :nblk * H])
                    o1 = pbb.tile([P, 4 * HC], bf16, tag="o1")
                    nc.vector.tensor_tensor(
                        out=_sub(o1[:], 0, [[HC, nblk], [C, H], [1, C]]),
                        in0=_sub(po[:], 0, [[R1, nblk], [C, H], [1, C]]),
                        in1=_sub(rde[:], 0, [[H, nblk], [1, H], [0, C]]),
                        op=mybir.AluOpType.mult)
                    nc.vector.tensor_tensor(
                        out=_sub(o1[:], 0, [[HC, nblk], [1, HC]]),
                        in0=_sub(o1[:], 0, [[HC, nblk], [1, HC]]),
                        in1=_sub(b1s[:], 0, [[0, nblk], [1, HC]]),
                        op=mybir.AluOpType.add)
                    nc.scalar.activation(out=o1[:, :nblk * HC],
                                         in_=o1[:, :nblk * HC],
                                         func=mybir.ActivationFunctionType.Relu)
                    # second layer projection per block (PE transpose path)
                    h2s8 = pbb.tile([P, 4 * R2], f32, tag="h2s8", bufs=1)
                    for bi, b in enumerate(sb["blocks"]):
                        ph2 = psh.tile([P, R2], f32, tag="ph2")
                        for k in range(NCK):
                            kk = min(P, HC - k * P)
                            ptr = pst.tile([P, P], bf16, tag="ptr")
                            nc.tensor.transpose(
                                out=ptr[:kk, :],
                                in_=o1[:, bi * HC + k * P:bi * HC + k * P + kk],
                                identity=idn[:])
                            rT = pbb.tile([P, P], bf16, tag="rT")
                            nc.vector.tensor_copy(out=rT[:kk, :], in_=ptr[:kk, :])
                            nc.tensor.matmul(out=ph2[:], lhsT=rT[:kk, :],
                                             rhs=w2s[k][:kk, :],
                                             start=(k == 0), stop=(k == NCK - 1))
                        nc.vector.tensor_copy(out=h2s8[:, bi * R2:(bi + 1) * R2],
                                              in_=ph2[:])
                    # batched stores: h2pad always full blocks; h2loc clipped
                    nc.sync.dma_start(
                        out=bass.AP(h2pad, b0 * P * R2,
                                    [[R2, P], [P * R2, nblk], [1, R2]]),
                        in_=_sub(h2s8[:], 0, [[R2, nblk], [1, R2]]))
                    nfull = sum(1 for b in sb["blocks"] if (b + 1) * P <= NPC)
                    if nfull:
                        nc.sync.dma_start(
                            out=bass.AP(h2loc, b0 * P * R2,
                                        [[R2, P], [P * R2, nfull], [1, R2]]),
                            in_=_sub(h2s8[:], 0, [[R2, nfull], [1, R2]]))
                    for bi, b in enumerate(sb["blocks"]):
                        if bi < nfull:
                            continue
                        rows = NPC - b * P
                        if rows > 0:
                            nc.sync.dma_start(
                                out=h2loc[b * P:b * P + rows, :],
                                in_=h2s8[:rows, bi * R2:(bi + 1) * R2])

            # ---------------- AllGather + repack -----------------------------
            nc.gpsimd.collective_compute(
                "AllGather", mybir.AluOpType.bypass,
                replica_groups=[list(range(NC))],
                ins=[h2loc[:, :]], outs=[h2tab[:, :]])
            # repack [N, R2] -> 256B f32 rows [N, RB2]
            for r in range(NC):
                nc.sync.dma_start(
                    out=bass.AP(h2tabp, r * NPC * RB2, [[RB2, NPC], [1, R2]]),
                    in_=h2tab[r * NPC:(r + 1) * NPC, :])

            # ---------------- Phase C: L2 edge pass --------------------------
            with tc.tile_pool(name="pcg", bufs=2) as pcg, \
                 tc.tile_pool(name="pcb", bufs=2) as pcb, \
                 tc.tile_pool(name="psc", bufs=2, space="PSUM") as psc, \
                 tc.tile_pool(name="psd2", bufs=2, space="PSUM") as psd2:
                for sb in sb_meta:
                    base, S = sb["base"], sb["S"]
                    nblk = len(sb["blocks"])
                    b0 = sb["blocks"][0]
                    g2 = pcg.tile([P, S * RB2], f32, tag="g2", bufs=3)
                    ixs = pcg.tile([P, S * 8], i16, tag="ixs2", bufs=3)
                    nc.scalar.dma_start(out=ixs[:],
                                        in_=ihsrc_d[:, base * 8:(base + S) * 8])
                    for q in range(NCHUNK):
                        tb, segT = sb["segs"][q]
                        if segT == 0:
                            continue
                        gather_split(g2, tb - base, segT, RB2,
                                     h2tabp[q * CHB:min(q * CHB + nchrows[q], N), :],
                                     ixs)
                    adw2 = pcg.tile([P, 8], bf16, tag="adw2")
                    nc.gpsimd.dma_start(
                        out=adw2[:, :nblk],
                        in_=bass.AP(h2pad, b0 * P * R2 + CLS + 1,
                                    [[R2, P], [P * R2, nblk], [1, 1]]))
                    oh2 = pcb.tile([P, S * P], bf16, tag="oh2", bufs=1)
                    nc.vector.tensor_tensor(
                        out=oh2[:].rearrange("p (t q) -> p t q", t=S),
                        in0=_sub(iot[:], 0, [[0, S], [1, P]]),
                        in1=_sub(dlc[:], base, [[1, S], [0, P]]),
                        op=mybir.AluOpType.is_equal)
                    dlR = pcg.tile([P, S * P], bf16, tag="dlR2")
                    nc.scalar.dma_start(out=dlR[:],
                                        in_=dlocR_d[:, base * P:(base + S) * P])
                    oTs = pcb.tile([P, S * P], bf16, tag="oTs2", bufs=1)
                    nc.vector.tensor_scalar(
                        out=oTs[:], in0=dlR[:], scalar1=0.0, scalar2=None,
                        op0=mybir.AluOpType.is_equal)
                    pad2 = psd2.tile([P, S], f32, tag="pad2")
                    for bi, b in enumerate(sb["blocks"]):
                        for (tg, tt) in sb["runs"][b]:
                            for t in range(tt):
                                rel = tg - base + t
                                nc.tensor.matmul(
                                    out=pad2[:, rel:rel + 1],
                                    lhsT=oTs[:, rel * P:(rel + 1) * P],
                                    rhs=adw2[:, bi:bi + 1],
                                    start=True, stop=True,
                                    skip_group_check=True)
                    ex2 = pcb.tile([P, S], f32, tag="ex2")
                    nc.vector.tensor_tensor(
                        out=ex2[:],
                        in0=_sub(g2[:], CLS, [[RB2, S]]),
                        in1=pad2[:],
                        op=mybir.AluOpType.add)
                    tm2 = pcb.tile([P, S], f32, tag="tm2")
                    nc.vector.tensor_scalar_mul(out=tm2[:], in0=ex2[:], scalar1=NEG)
                    nc.vector.tensor_tensor(out=ex2[:], in0=ex2[:], in1=tm2[:],
                                            op=mybir.AluOpType.max)
                    nc.scalar.activation(out=ex2[:], in_=ex2[:],
                                         func=mybir.ActivationFunctionType.Exp)
                    m2 = pcb.tile([P, S * 4], bf16, tag="m2")
                    nc.vector.tensor_copy(out=_sub(m2[:], CLS, [[4, S]]), in_=ex2[:])
                    nc.vector.tensor_copy(out=_sub(m2[:], CLS + 1, [[4, S]]),
                                          in_=ex2[:])
                    nc.vector.tensor_tensor(
                        out=_sub(m2[:], 0, [[4, S], [1, CLS]]),
                        in0=_sub(g2[:], 0, [[RB2, S], [1, CLS]]),
                        in1=_sub(m2[:], CLS, [[4, S], [0, CLS]]),
                        op=mybir.AluOpType.mult)
                    v0 = pcb.tile([P, 4 * R2], f32, tag="v0", bufs=1)
                    for bi, b in enumerate(sb["blocks"]):
                        runs = sb["runs"][b]
                        ntile = sum(t for _, t in runs)
                        ps2 = psc.tile([P, 4], f32, tag="ps2")
                        ti = 0
                        for (tg, tt) in runs:
                            for t in range(tt):
                                rel = tg - base + t
                                nc.tensor.matmul(
                                    out=ps2[:],
                                    lhsT=oh2[:, rel * P:(rel + 1) * P],
                                    rhs=m2[:, rel * 4:(rel + 1) * 4],
                                    start=(ti == 0), stop=(ti == ntile - 1))
                                ti += 1
                        nc.vector.tensor_copy(out=v0[:, bi * R2:(bi + 1) * R2],
                                              in_=ps2[:])
                    # batched normalize + bias + log_softmax over the sb
                    dn2 = pcb.tile([P, 4], f32, tag="dn2")
                    nc.vector.tensor_scalar_max(
                        out=dn2[:, :nblk],
                        in0=_sub(v0[:], CLS, [[R2, nblk]]),
                        scalar1=1e-20)
                    rd2 = pcb.tile([P, 4], f32, tag="rd2")
                    nc.vector.reciprocal(out=rd2[:, :nblk], in_=dn2[:, :nblk])
                    vv = pcb.tile([P, 4 * CLS], f32, tag="vv")
                    nc.vector.tensor_tensor(
                        out=_sub(vv[:], 0, [[CLS, nblk], [1, CLS]]),
                        in0=_sub(v0[:], 0, [[R2, nblk], [1, CLS]]),
                        in1=_sub(rd2[:], 0, [[1, nblk], [0, CLS]]),
                        op=mybir.AluOpType.mult)
                    nc.vector.tensor_tensor(
                        out=_sub(vv[:], 0, [[CLS, nblk], [1, CLS]]),
                        in0=_sub(vv[:], 0, [[CLS, nblk], [1, CLS]]),
                        in1=_sub(b2s[:], 0, [[0, nblk], [1, CLS]]),
                        op=mybir.AluOpType.add)
                    mx = pcb.tile([P, 4], f32, tag="mx")
                    nc.vector.tensor_reduce(
                        out=_sub(mx[:], 0, [[1, nblk]]),
                        in_=_sub(vv[:], 0, [[CLS, nblk], [1, CLS]]),
                        axis=mybir.AxisListType.X,
                        op=mybir.AluOpType.max)
                    u = pcb.tile([P, 4 * CLS], f32, tag="u")
                    nc.vector.tensor_tensor(
                        out=_sub(u[:], 0, [[CLS, nblk], [1, CLS]]),
                        in0=_sub(vv[:], 0, [[CLS, nblk], [1, CLS]]),
                        in1=_sub(mx[:], 0, [[1, nblk], [0, CLS]]),
                        op=mybir.AluOpType.subtract)
                    nc.scalar.activation(out=u[:, :nblk * CLS],
                                         in_=u[:, :nblk * CLS],
                                         func=mybir.ActivationFunctionType.Exp)
                    sm = pcb.tile([P, 4], f32, tag="sm")
                    nc.vector.tensor_reduce(
                        out=_sub(sm[:], 0, [[1, nblk]]),
                        in_=_sub(u[:], 0, [[CLS, nblk], [1, CLS]]),
                        axis=mybir.AxisListType.X,
                        op=mybir.AluOpType.add)
                    ls = pcb.tile([P, 4], f32, tag="ls")
                    nc.scalar.activation(out=ls[:, :nblk], in_=sm[:, :nblk],
                                         func=mybir.ActivationFunctionType.Ln)
                    nc.vector.tensor_tensor(out=ls[:, :nblk], in0=ls[:, :nblk],
                                            in1=mx[:, :nblk],
                                            op=mybir.AluOpType.add)
                    res = pcb.tile([P, 4 * CLS], f32, tag="res")
                    nc.vector.tensor_tensor(
                        out=_sub(res[:], 0, [[CLS, nblk], [1, CLS]]),
                        in0=_sub(vv[:], 0, [[CLS, nblk], [1, CLS]]),
                        in1=_sub(ls[:], 0, [[1, nblk], [0, CLS]]),
                        op=mybir.AluOpType.subtract)
                    nfull = sum(1 for b in sb["blocks"] if (b + 1) * P <= NPC)
                    if nfull:
                        nc.sync.dma_start(
                            out=bass.AP(out_d, b0 * P * CLS,
                                        [[CLS, P], [P * CLS, nfull], [1, CLS]]),
                            in_=_sub(res[:], 0, [[CLS, nfull], [1, CLS]]))
                    for bi, b in enumerate(sb["blocks"]):
                        if bi < nfull:
                            continue
                        rows = NPC - b * P
                        if rows > 0:
                            nc.sync.dma_start(
                                out=out_d[b * P:b * P + rows, :],
                                in_=res[:rows, bi * CLS:(bi + 1) * CLS])
            _pa_stack.close()
    nc.finalize()
    return nc


def install_ntff_hook(so_path="/opt/axon/libaxon_pjrt.so"):
    import types
    import ctypes
    import contextlib
    import antenv

    if getattr(antenv, "axon_hooks", None) is not None:
        return
    lib = ctypes.CDLL(so_path)
    if not hasattr(lib, "axon_start_nrt_profile"):
        return
    lib.axon_start_nrt_profile.argtypes = [ctypes.POINTER(ctypes.c_int64),
                                           ctypes.c_size_t]
    lib.axon_start_nrt_profile.restype = ctypes.c_int64
    lib.axon_stop_nrt_profile.argtypes = [ctypes.c_char_p]
    lib.axon_stop_nrt_profile.restype = ctypes.c_int64

    @contextlib.contextmanager
    def _hook(output_dir, device_ids):
        import jax
        jax.devices()
        if device_ids:
            ids = (ctypes.c_int64 * len(device_ids))(*device_ids)
            rc = lib.axon_start_nrt_profile(ids, len(device_ids))
        else:
            rc = lib.axon_start_nrt_profile(None, 0)
        if rc != 0:
            raise RuntimeError(f"axon_start_nrt_profile rc={rc}")
        try:
            yield
        finally:
            n = lib.axon_stop_nrt_profile(str(output_dir).encode())
            print(f"ntff profile: {n} file(s) written to {output_dir}")

    mod = types.ModuleType("antenv.axon_hooks")
    _reg = [_hook]
    mod.set_axon_ntff_profile_hook = lambda h: _reg.__setitem__(0, h)
    mod.get_axon_ntff_profile_hook = lambda: _reg[0]
    sys.modules["antenv.axon_hooks"] = mod
    antenv.axon_hooks = mod


def run(inputs, cfg, trace=False, **kwargs):
    if trace:
        install_ntff_hook()
    in_maps, meta = prep(inputs, cfg)
    nc = build(meta)
    res = bass_utils.run_bass_kernel_spmd(
        nc, in_maps, core_ids=list(range(cfg["NC"])), trace=trace, **kwargs)
    out = np.concatenate([res.results[c]["out"] for c in range(cfg["NC"])], axis=0)
    return out, res


# ----------------------------------------------------------------------------
# harness entry point
# ----------------------------------------------------------------------------

_CFG = dict(N=100000, F=165, H=4, C=64, CLS=2, NC=8)


def kernel(**inputs):
    """Full (unsharded) inputs -> full [N, 2] float32 log-softmax output.

    Shards edges by destination-node range across the 8 NeuronCores,
    compiles and runs the Bass/Tile kernel via run_bass_kernel_spmd,
    and concatenates the per-core output slices.
    """
    out, _ = run(inputs, _CFG, trace=False)
    return np.ascontiguousarray(out.astype(np.float32))


# revision 61
# speedup vs baseline: 1.0643x; 1.0643x over previous
"""GAT 2-layer message-passing network on 8 TRN2 NeuronCores (Bass/Tile).

v3: dispatch/instruction-count optimized.

Strategy (dst-sharded):
 - Host: add self loops, sort edges by dst, shard dst-node ranges across cores.
   Each core owns nodes [c*NPC, (c+1)*NPC) and ALL edges into them.
 - Edge slots: per dst-block of 128 nodes, edges sub-grouped by src chunk
   (4 chunks of CH rows so int16 indices work), each (block,chunk) run padded
   to x128 slots = tiles. Superblocks of SBG blocks share gather calls.
 - Phase A (replicated): full feature table htab[n] = [h|a_src|pad] bf16
   [Np, 384] (768B rows for dma_gather), + local stats table sloc
   [NPCp, H] bf16 (a_dst of the core's own nodes), batched 8 tiles per DMA.
 - Phase B (L1): per sb: dma_gather htab rows by src (4 chunk calls);
   oT one-hot ([dst, slot]) via is_equal against the host-replicated
   dlocR table (no PE broadcast); per-tile matmuls oT x adw -> per-edge
   a_dst; ex = exp(lrelu(asrc+adst)) batched per sb; msg in-place in the
   gather buffer; one-hot oh from dloc2d vs iota; per-block PSUM matmul
   accumulation; per-sb batched normalize + b1 + relu; h2aug = relu @ W2aug
   via PE transpose; batched h2loc (AG input) + h2pad stores.
 - AllGather h2loc -> h2tab [N,4] f32; repack into h2tabp [Npp, 64] f32 rows.
 - Phase C (L2): same slots: gather h2tabp by src; same oT/oh structure;
   4-wide bf16 messages; one-hot matmuls; per-sb batched normalize, +b2,
   log_softmax -> out [NPC, 2] f32.
"""
import sys

if "/opt/trn_rl_repo" not in sys.path:
    sys.path.insert(0, "/opt/trn_rl_repo")

import math
import numpy as np
import ml_dtypes

import concourse.bass as bass
import concourse.bacc as bacc
import concourse.mybir as mybir
import concourse.tile as tile
from concourse import bass_utils

P = 128
NEG = 0.2
NCHUNK = 4
NQUEUE = 4
BAT = 8                      # phase-A tiles per DMA batch

# Tile's DMA sem-lane assignment round-robins over all DMAs of a DGE class,
# which breaks the per-lane FIFO assumption when DMAs run on multiple HW
# rings (out-of-order completion across rings under one counting sem):
#  - SWDGE (Pool) on multiple queues -> lane == queue_num.
#  - HWDGE from both SP (sync) and ACT (scalar) rings -> SP lanes 0-3,
#    ACT lanes 4-7 (per-engine round-robin).
from concourse import tile_sem_assignment as _tsa  # noqa: E402

if not getattr(_tsa.TileClockTick, "_qaware_patched", False):
    _orig_assign_tick = _tsa.TileClockTick._assign_tick

    def _qaware_assign_tick(self, inst):
        if isinstance(inst, _tsa.DMAInst):
            if inst.engine == mybir.EngineType.Pool:
                self.next_sw_dma_idx = getattr(inst, "queue_num", 0) or 0
            elif inst.engine in (mybir.EngineType.SP,
                                 mybir.EngineType.Activation):
                if not hasattr(self, "_hw_rr"):
                    self._hw_rr = [0, 0]
                w = 1 if inst.engine == mybir.EngineType.Activation else 0
                self.next_hw_dma_idx = w * 4 + (self._hw_rr[w] % 4)
                self._hw_rr[w] += 1
        return _orig_assign_tick(self, inst)

    _tsa.TileClockTick._assign_tick = _qaware_assign_tick
    _tsa.TileClockTick._qaware_patched = True


def _wrap16(flat):
    """[n] -> [128, n//16] wrapped in 16 partitions, replicated x8."""
    w = flat.reshape(-1, 16).T
    return np.tile(w, (8, 1))


# ----------------------------------------------------------------------------
# host-side data prep
# ----------------------------------------------------------------------------

def prep(inputs, cfg):
    N, F, H, C, CLS, NC = cfg["N"], cfg["F"], cfg["H"], cfg["C"], cfg["CLS"], cfg["NC"]
    SBG = cfg.get("SBG", 4)
    x = np.asarray(inputs["x"], np.float32)
    ei = np.asarray(inputs["edge_index"])
    W1 = np.asarray(inputs["W1"], np.float32)
    as1 = np.asarray(inputs["att_src1"], np.float32)
    ad1 = np.asarray(inputs["att_dst1"], np.float32)
    b1 = np.asarray(inputs["b1"], np.float32)
    W2 = np.asarray(inputs["W2"], np.float32)
    as2 = np.asarray(inputs["att_src2"], np.float32)
    ad2 = np.asarray(inputs["att_dst2"], np.float32)
    b2 = np.asarray(inputs["b2"], np.float32)

    HC = H * C
    R1 = HC + 2 * H                      # live row payload [h | asrc | adst]
    RG = 128 * math.ceil(R1 / 128)       # htab gather row elems (bf16, 256B mult)
    NPC = N // NC
    NB = math.ceil(NPC / P)
    NPCp = NB * P                        # padded local rows
    NT = (N + P - 1) // P
    Np = NT * P
    # chunk base, tile-aligned so phase-A stores land in per-chunk ranges
    # (lets chunk-q gathers start before the whole table is written)
    CHB = math.ceil(N / NCHUNK / P) * P
    assert Np - (NCHUNK - 1) * CHB < 32768  # last-chunk slice
    assert CHB < 32768 and NPCp < 32768

    # ---- weights / constants -------------------------------------------------
    W1r = W1.reshape(F, H, C)
    Wsrc = np.einsum("fhc,hc->fh", W1r, as1)
    Wdst = np.einsum("fhc,hc->fh", W1r, ad1)
    W1aug = np.concatenate([W1, Wsrc, Wdst], axis=1)          # [F, R1]
    Wsrc2 = W2 @ as2.reshape(CLS, 1)
    Wdst2 = W2 @ ad2.reshape(CLS, 1)
    W2aug = np.concatenate([W2, Wsrc2, Wdst2], axis=1)        # [HC, 4]

    bf16 = ml_dtypes.bfloat16
    xT = np.zeros((F, Np), dtype=bf16)
    xT[:, :N] = x.T.astype(bf16)
    W1aug_b = W1aug.astype(bf16)
    W2aug_b = W2aug.astype(bf16)
    b1rep = np.tile(b1[None, :], (P, 1)).astype(bf16)
    b2rep = np.tile(b2[None, :], (P, 1)).astype(np.float32)
    iota = np.tile(np.arange(P, dtype=np.float32)[None, :], (P, 1)).astype(bf16)
    ident = np.eye(P, dtype=bf16)

    # ---- edges ---------------------------------------------------------------
    src_all = np.concatenate([ei[0], np.arange(N, dtype=ei.dtype)]).astype(np.int64)
    dst_all = np.concatenate([ei[1], np.arange(N, dtype=ei.dtype)]).astype(np.int64)
    order = np.argsort(dst_all, kind="stable")
    src_s = src_all[order]
    dst_s = dst_all[order]
    chunk_s = src_s // CHB

    cnts = np.zeros((NC, NB, NCHUNK), np.int64)
    for c in range(NC):
        for b in range(NB):
            base = c * NPC + b * P
            hi = min(base + P, (c + 1) * NPC)
            lo_i = np.searchsorted(dst_s, base)
            hi_i = np.searchsorted(dst_s, hi)
            ch = chunk_s[lo_i:hi_i]
            for q in range(NCHUNK):
                cnts[c, b, q] = (ch == q).sum()
    Trun = np.ceil(cnts / P).astype(np.int64).max(axis=0)     # [NB, NCHUNK]

    # superblocks
    sblocks = [list(range(i, min(i + SBG, NB))) for i in range(0, NB, SBG)]
    # slot layout: per sb: for q: for b in sb: Trun[b,q] tiles
    sb_meta = []
    tile_base = 0
    for blist in sblocks:
        segs = []           # per q: (seg_tile_base_global, segT)
        runs = {b: [] for b in blist}   # block -> [(tile_global, T)]
        sb_base = tile_base
        for q in range(NCHUNK):
            segT = int(Trun[blist, q].sum())
            segs.append((tile_base, segT))
            tb = tile_base
            for b in blist:
                t = int(Trun[b, q])
                if t:
                    runs[b].append((tb, t))
                tb += t
            tile_base += segT
        sb_meta.append(dict(base=sb_base, S=tile_base - sb_base, segs=segs,
                            blocks=blist, runs=runs))
    Tsum = tile_base

    # per-core slot-value arrays
    ihsrc_w = np.zeros((NC, P, Tsum * 8), np.int16)
    dloc2d = np.full((NC, P, Tsum), 255.0, bf16)
    dlocR_a = np.zeros((NC, P, Tsum * P), bf16)
    for c in range(NC):
        ihsrc = np.zeros(Tsum * P, np.int16)
        dloc = np.full(Tsum * P, 255.0, np.float32)
        core_lo = np.searchsorted(dst_s, c * NPC)
        core_hi = np.searchsorted(dst_s, (c + 1) * NPC)
        cs, cd, cq = (src_s[core_lo:core_hi], dst_s[core_lo:core_hi],
                      chunk_s[core_lo:core_hi])
        # edges sorted by (dst, chunk); regroup per (block, chunk)
        for sb in sb_meta:
            for q in range(NCHUNK):
                for b in sb["blocks"]:
                    t = int(Trun[b, q])
                    if t == 0:
                        continue
                    # this block+chunk's edges (mask within the dst range)
                    base = c * NPC + b * P
                    hi = min(base + P, (c + 1) * NPC)
                    seg = slice(np.searchsorted(cd, base), np.searchsorted(cd, hi))
                    m = cq[seg] == q
                    es, ed = cs[seg][m], cd[seg][m]
                    n = len(es)
                    assert n <= t * P
                    # locate this run's global tile index (runs are in q order)
                    tg = None
                    for (tgi, tti) in sb["runs"][b]:
                        s0, sT = sb["segs"][q]
                        if s0 <= tgi < s0 + sT:
                            tg = tgi
                            break
                    assert tg is not None
                    s0 = tg * P
                    ihsrc[s0:s0 + n] = (es - q * CHB).astype(np.int16)
                    dloc[s0:s0 + n] = (ed - (c * NPC + b * P)).astype(np.float32)
        ihsrc_w[c] = _wrap16(ihsrc)
        dloc2d[c] = dloc.reshape(Tsum, P).T.astype(bf16)
        # dloc[s] - p per partition: oT = (dlz == 0) via fast tensor_scalar
        dlocR_a[c] = (dloc[None, :]
                      - np.arange(P, dtype=np.float32)[:, None]).astype(bf16)

    shared = {
        "xT": xT, "W1aug": W1aug_b, "W2aug": W2aug_b, "b1rep": b1rep,
        "b2rep": b2rep, "iota": iota, "ident": ident,
    }
    in_maps = []
    for c in range(NC):
        m = dict(shared)
        xl = np.zeros((F, NPCp), dtype=bf16)
        xl[:, :NPC] = xT[:, c * NPC:c * NPC + NPC]
        m["xTloc"] = xl
        m["ihsrc"] = ihsrc_w[c]
        m["dloc2d"] = dloc2d[c]
        m["dlocR"] = dlocR_a[c]
        in_maps.append(m)

    meta = dict(cfg, R1=R1, RG=RG, HC=HC, NPC=NPC, NPCp=NPCp, NB=NB, NT=NT,
                Np=Np, CHB=CHB, Tsum=Tsum, sb_meta=sb_meta, SBG=SBG)
    return in_maps, meta


# ----------------------------------------------------------------------------
# device program
# ----------------------------------------------------------------------------

def _sub(ap, elem_off, dims):
    return bass.AP(ap.tensor, ap.offset + elem_off, [ap.ap[0], *list(dims)])


def build(meta, nc=None):
    N, F, H, C, CLS = meta["N"], meta["F"], meta["H"], meta["C"], meta["CLS"]
    NC, R1, RG, HC = meta["NC"], meta["R1"], meta["RG"], meta["HC"]
    NPC, NPCp, NB, NT, Np = (meta["NPC"], meta["NPCp"], meta["NB"], meta["NT"],
                             meta["Np"])
    CHB, Tsum = meta["CHB"], meta["Tsum"]
    sb_meta = meta["sb_meta"]
    R2 = CLS + 2
    RB2 = 64                           # f32 row elems for L2 gather tables

    f32, bf16, i16 = mybir.dt.float32, mybir.dt.bfloat16, mybir.dt.int16

    if nc is None:
        nc = bacc.Bacc("TRN2", target_bir_lowering=False, debug=False,
                       num_devices=NC, num_swdge_queues=NQUEUE)

    MAXT = 7                 # tiles per dma_gather call (<=896 descs, carveout 1024)
    qrr = [0]

    def gather_split(out_tile, rel, segT, elem, table, ix_tile):
        """Split a segment gather into <=MAXT-tile calls, round-robin queues."""
        done = 0
        while done < segT:
            tt = min(MAXT, segT - done)
            r = rel + done
            nc.gpsimd.dma_gather(
                bass.AP(out_tile[:].tensor, out_tile[:].offset + r * elem,
                        [out_tile[:].ap[0], [elem, tt], [1, elem]]),
                table,
                ix_tile[:, r * 8:(r + tt) * 8],
                tt * P, tt * P, elem,
                queue_num=qrr[0] % NQUEUE,
            )
            qrr[0] += 1
            done += tt

    xT_d = nc.dram_tensor("xT", [F, Np], bf16, kind="ExternalInput")
    xTl_d = nc.dram_tensor("xTloc", [F, NPCp], bf16, kind="ExternalInput")
    W1aug_d = nc.dram_tensor("W1aug", [F, R1], bf16, kind="ExternalInput")
    W2aug_d = nc.dram_tensor("W2aug", [HC, R2], bf16, kind="ExternalInput")
    b1rep_d = nc.dram_tensor("b1rep", [P, HC], bf16, kind="ExternalInput")
    b2rep_d = nc.dram_tensor("b2rep", [P, CLS], f32, kind="ExternalInput")
    iota_d = nc.dram_tensor("iota", [P, P], bf16, kind="ExternalInput")
    ident_d = nc.dram_tensor("ident", [P, P], bf16, kind="ExternalInput")
    ihsrc_d = nc.dram_tensor("ihsrc", [P, Tsum * 8], i16, kind="ExternalInput")
    dloc_d = nc.dram_tensor("dloc2d", [P, Tsum], bf16, kind="ExternalInput")
    dlocR_d = nc.dram_tensor("dlocR", [P, Tsum * P], bf16, kind="ExternalInput")
    out_d = nc.dram_tensor("out", [NPC, CLS], f32, kind="ExternalOutput")

    # per-chunk h tables (separate tensors so chunk-q gathers only depend on
    # chunk-q phase-A stores)
    nchrows = [CHB] * (NCHUNK - 1) + [Np - (NCHUNK - 1) * CHB]
    htabq = [nc.dram_tensor(f"htab{q}", [nchrows[q], RG], bf16, kind="Internal")
             for q in range(NCHUNK)]
    sloc = nc.dram_tensor("sloc", [NPCp, H], bf16, kind="Internal")
    h2loc = nc.dram_tensor("h2loc", [NPC, R2], f32, kind="Internal")
    h2pad = nc.dram_tensor("h2pad", [NPCp, R2], f32, kind="Internal")
    h2tab = nc.dram_tensor("h2tab", [N, R2], f32, kind="Internal",
                           addr_space="Shared" if NC > 4 else "Local")
    h2tabp = nc.dram_tensor("h2tabp", [N, RB2], f32, kind="Internal")

    FA = min(P, F)
    FB = F - FA
    NCK = (HC + P - 1) // P

    with tile.TileContext(nc) as tc:
        with tc.tile_pool(name="const", bufs=1) as cp:
            w1a = cp.tile([FA, R1], bf16)
            nc.sync.dma_start(out=w1a[:], in_=W1aug_d[0:FA, :])
            if FB:
                w1b = cp.tile([FB, R1], bf16)
                nc.sync.dma_start(out=w1b[:], in_=W1aug_d[FA:F, :])
            w2s = []
            for k in range(NCK):
                kk = min(P, HC - k * P)
                w2k = cp.tile([kk, R2], bf16, name=f"w2k{k}")
                nc.sync.dma_start(out=w2k[:], in_=W2aug_d[k * P:k * P + kk, :])
                w2s.append(w2k)
            b1s = cp.tile([P, HC], bf16)
            nc.sync.dma_start(out=b1s[:], in_=b1rep_d[:, :])
            b2s = cp.tile([P, CLS], f32)
            nc.sync.dma_start(out=b2s[:], in_=b2rep_d[:, :])
            iot = cp.tile([P, P], bf16)
            nc.sync.dma_start(out=iot[:], in_=iota_d[:, :])
            idn = cp.tile([P, P], bf16)
            nc.sync.dma_start(out=idn[:], in_=ident_d[:, :])
            dlc = cp.tile([P, Tsum], bf16)
            nc.sync.dma_start(out=dlc[:], in_=dloc_d[:, :])

            # ---------------- Phase A: feature tables ------------------------
            # local a_dst stats FIRST so phase-B adw loads unblock early.
            # pa (SBUF) stays open through B/C so phase-B tiles don't reuse
            # its addresses (address-reuse WAR would chain gathers behind
            # the whole of phase A); only the PSUM pool closes.
            import contextlib
            _pa_stack = contextlib.ExitStack()
            pa = _pa_stack.enter_context(tc.tile_pool(name="pa", bufs=2))
            with tc.tile_pool(name="psa", bufs=4, space="PSUM") as psa:
                for g8 in range(0, NPCp // P, BAT):
                    nb8 = min(BAT, NPCp // P - g8)
                    xa = pa.tile([FA, BAT * P], bf16, tag="xla")
                    nc.sync.dma_start(out=xa[:, :nb8 * P],
                                      in_=xTl_d[0:FA, g8 * P:(g8 + nb8) * P])
                    if FB:
                        xb = pa.tile([FB, BAT * P], bf16, tag="xlb")
                        nc.sync.dma_start(out=xb[:, :nb8 * P],
                                          in_=xTl_d[FA:F, g8 * P:(g8 + nb8) * P])
                    ss8 = pa.tile([P, BAT * H], bf16, tag="ss8")
                    for k in range(nb8):
                        ps = psa.tile([P, H], f32, tag="pss")
                        nc.tensor.matmul(out=ps[:], lhsT=xa[:, k * P:(k + 1) * P],
                                         rhs=w1a[:, HC + H:HC + 2 * H],
                                         start=True, stop=(FB == 0))
                        if FB:
                            nc.tensor.matmul(out=ps[:],
                                             lhsT=xb[:, k * P:(k + 1) * P],
                                             rhs=w1b[:, HC + H:HC + 2 * H],
                                             start=False, stop=True)
                        nc.vector.tensor_copy(out=ss8[:, k * H:(k + 1) * H],
                                              in_=ps[:])
                    nc.sync.dma_start(
                        out=bass.AP(sloc, g8 * P * H,
                                    [[H, P], [P * H, nb8], [1, H]]),
                        in_=_sub(ss8[:], 0, [[H, nb8], [1, H]]))
                for q in range(NCHUNK):
                    qt0 = q * CHB // P
                    qnt = min(NT, (q * CHB + nchrows[q]) // P) - qt0
                    for g8 in range(qt0, qt0 + qnt, BAT):
                        nb8 = min(BAT, qt0 + qnt - g8)
                        xa = pa.tile([FA, BAT * P], bf16, tag="xa")
                        nc.sync.dma_start(out=xa[:, :nb8 * P],
                                          in_=xT_d[0:FA, g8 * P:(g8 + nb8) * P])
                        if FB:
                            xb = pa.tile([FB, BAT * P], bf16, tag="xb")
                            nc.sync.dma_start(
                                out=xb[:, :nb8 * P],
                                in_=xT_d[FA:F, g8 * P:(g8 + nb8) * P])
                        hs8 = pa.tile([P, BAT * R1], bf16, tag="hs8")
                        for k in range(nb8):
                            ph = psa.tile([P, R1], f32, tag="ph")
                            nc.tensor.matmul(out=ph[:],
                                             lhsT=xa[:, k * P:(k + 1) * P],
                                             rhs=w1a[:], start=True,
                                             stop=(FB == 0))
                            if FB:
                                nc.tensor.matmul(out=ph[:],
                                                 lhsT=xb[:, k * P:(k + 1) * P],
                                                 rhs=w1b[:], start=False,
                                                 stop=True)
                            nc.vector.tensor_copy(
                                out=hs8[:, k * R1:(k + 1) * R1], in_=ph[:])
                        nc.sync.dma_start(
                            out=bass.AP(htabq[q], (g8 - qt0) * P * RG,
                                        [[RG, P], [P * RG, nb8], [1, R1]]),
                            in_=_sub(hs8[:], 0, [[R1, nb8], [1, R1]]))

            # ---------------- Phase B: L1 edge pass --------------------------
            with tc.tile_pool(name="pbg", bufs=2) as pbg, \
                 tc.tile_pool(name="pbb", bufs=2) as pbb, \
                 tc.tile_pool(name="psb", bufs=2, space="PSUM") as psb, \
                 tc.tile_pool(name="pst", bufs=2, space="PSUM") as pst, \
                 tc.tile_pool(name="psh", bufs=2, space="PSUM") as psh, \
                 tc.tile_pool(name="psa2", bufs=2, space="PSUM") as psa2:
                for sb in sb_meta:
                    base, S = sb["base"], sb["S"]
                    nblk = len(sb["blocks"])
                    b0 = sb["blocks"][0]
                    g = pbg.tile([P, S * RG], bf16, tag="g")
                    ixs = pbg.tile([P, S * 8], i16, tag="ixs")
                    nc.scalar.dma_start(out=ixs[:],
                                        in_=ihsrc_d[:, base * 8:(base + S) * 8])
                    for q in range(NCHUNK):
                        tb, segT = sb["segs"][q]
                        if segT == 0:
                            continue
                        gather_split(g, tb - base, segT, RG, htabq[q][:, :], ixs)
                    # a_dst window for the sb's blocks  [P, nblk*H] bf16
                    adw = pbg.tile([P, 8 * H], bf16, tag="adw")
                    nc.scalar.dma_start(
                        out=adw[:, :nblk * H],
                        in_=bass.AP(sloc, b0 * P * H,
                                    [[H, P], [P * H, nblk], [1, H]]))
                    # one-hot for all slots  [P, S*P] bf16 (two halves)
                    oh = pbb.tile([P, S * P], bf16, tag="oh", bufs=1)
                    OH2 = (S + 1) // 2
                    for z0 in range(0, S, OH2):
                        nz = min(OH2, S - z0)
                        nc.vector.tensor_tensor(
                            out=_sub(oh[:], z0 * P, [[P, nz], [1, P]]),
                            in0=_sub(iot[:], 0, [[0, nz], [1, P]]),
                            in1=_sub(dlc[:], base + z0, [[1, nz], [0, P]]),
                            op=mybir.AluOpType.is_equal)
                    # O_T: [d, slot] one-hot via host (dloc - p) table
                    dlR = pbg.tile([P, S * P], bf16, tag="dlR")
                    nc.scalar.dma_start(out=dlR[:],
                                        in_=dlocR_d[:, base * P:(base + S) * P])
                    oTs = pbb.tile([P, S * P], bf16, tag="oTs", bufs=1)
                    nc.vector.tensor_scalar(
                        out=oTs[:], in0=dlR[:], scalar1=0.0, scalar2=None,
                        op0=mybir.AluOpType.is_equal)
                    # per-edge a_dst: oT x adw matmuls -> PSUM [P, S*H]
                    pad = psa2.tile([P, S * H], f32, tag="pad")
                    for bi, b in enumerate(sb["blocks"]):
                        for (tg, tt) in sb["runs"][b]:
                            for t in range(tt):
                                rel = tg - base + t
                                nc.tensor.matmul(
                                    out=pad[:, rel * H:(rel + 1) * H],
                                    lhsT=oTs[:, rel * P:(rel + 1) * P],
                                    rhs=adw[:, bi * H:(bi + 1) * H],
                                    start=True, stop=True,
                                    skip_group_check=True)
                    # ex = exp(lrelu(asrc+adst)), msg multiply and denom
                    # copies, all SEGMENT-aligned: each chunk-segment's chain
                    # depends only on its own gather, so segment-0 compute
                    # overlaps segments 1-3 still draining.
                    ex = pbb.tile([P, S * H], f32, tag="ex", bufs=1)
                    tmp = pbb.tile([P, S * H], f32, tag="tmp", bufs=1)
                    exb = pbb.tile([P, S * H], bf16, tag="exb", bufs=1)
                    for qi in range(NCHUNK):
                        tbq, segT = sb["segs"][qi]
                        if segT == 0:
                            continue
                        s0 = tbq - base
                        ns = segT
                        exv = ex[:, s0 * H:(s0 + ns) * H]
                        tmv = tmp[:, s0 * H:(s0 + ns) * H]
                        nc.vector.tensor_tensor(
                            out=_sub(ex[:], s0 * H, [[H, ns], [1, H]]),
                            in0=_sub(g[:], s0 * RG + HC, [[RG, ns], [1, H]]),
                            in1=_sub(pad[:], s0 * H, [[H, ns], [1, H]]),
                            op=mybir.AluOpType.add)
                        nc.vector.tensor_scalar_mul(out=tmv, in0=exv,
                                                    scalar1=NEG)
                        nc.vector.tensor_tensor(out=exv, in0=exv, in1=tmv,
                                                op=mybir.AluOpType.max)
                        nc.scalar.activation(
                            out=exv, in_=exv,
                            func=mybir.ActivationFunctionType.Exp)
                        nc.vector.tensor_copy(
                            out=exb[:, s0 * H:(s0 + ns) * H], in_=exv)
                        nc.vector.tensor_tensor(
                            out=_sub(g[:], s0 * RG, [[RG, ns], [C, H], [1, C]]),
                            in0=_sub(g[:], s0 * RG, [[RG, ns], [C, H], [1, C]]),
                            in1=_sub(exb[:], s0 * H, [[H, ns], [1, H], [0, C]]),
                            op=mybir.AluOpType.mult)
                        nc.vector.tensor_copy(
                            out=_sub(g[:], s0 * RG + HC, [[RG, ns], [1, H]]),
                            in_=_sub(exb[:], s0 * H, [[H, ns], [1, H]]))
                        nc.vector.tensor_copy(
                            out=_sub(g[:], s0 * RG + HC + H, [[RG, ns], [1, H]]),
                            in_=_sub(exb[:], s0 * H, [[H, ns], [1, H]]))
                    # per-block accumulation
                    po = pbb.tile([P, 4 * R1], f32, tag="po", bufs=1)
                    for bi, b in enumerate(sb["blocks"]):
                        runs = sb["runs"][b]
                        ntile = sum(t for _, t in runs)
                        pso = psb.tile([P, R1], f32, tag="pso")
                        ti = 0
                        for (tg, tt) in runs:
                            for t in range(tt):
                                rel = tg - base + t
                                nc.tensor.matmul(
                                    out=pso[:],
                                    lhsT=oh[:, rel * P:(rel + 1) * P],
                                    rhs=g[:, rel * RG:rel * RG + R1],
                                    start=(ti == 0), stop=(ti == ntile - 1))
                                ti += 1
                        nc.vector.tensor_copy(out=po[:, bi * R1:(bi + 1) * R1],
                                              in_=pso[:])
                    # batched normalize + bias + relu over the sb's blocks
                    den = pbb.tile([P, 4 * H], f32, tag="den")
                    nc.vector.tensor_scalar_max(
                        out=den[:, :nblk * H],
                        in0=_sub(po[:], HC, [[R1, nblk], [1, H]]),
                        scalar1=1e-20)
                    rde = pbb.tile([P, 4 * H], f32, tag="rde")
                    nc.vector.reciprocal(out=rde[:, :nblk * H],
                                         in_=den[:, :nblk * H])
                    o1 = pbb.tile([P, 4 * HC], bf16, tag="o1")
                    nc.vector.tensor_tensor(
                        out=_sub(o1[:], 0, [[HC, nblk], [C, H], [1, C]]),
                        in0=_sub(po[:], 0, [[R1, nblk], [C, H], [1, C]]),
                        in1=_sub(rde[:], 0, [[H, nblk], [1, H], [0, C]]),
                        op=mybir.AluOpType.mult)
                    nc.vector.tensor_tensor(
                        out=_sub(o1[:], 0, [[HC, nblk], [1, HC]]),
                        in0=_sub(o1[:], 0, [[HC, nblk], [1, HC]]),
                        in1=_sub(b1s[:], 0, [[0, nblk], [1, HC]]),
                        op=mybir.AluOpType.add)
                    nc.scalar.activation(out=o1[:, :nblk * HC],
                                         in_=o1[:, :nblk * HC],
                                         func=mybir.ActivationFunctionType.Relu)
                    # second layer projection per block (PE transpose path)
                    h2s8 = pbb.tile([P, 4 * R2], f32, tag="h2s8", bufs=1)
                    for bi, b in enumerate(sb["blocks"]):
                        ph2 = psh.tile([P, R2], f32, tag="ph2")
                        for k in range(NCK):
                            kk = min(P, HC - k * P)
                            ptr = pst.tile([P, P], bf16, tag="ptr")
                            nc.tensor.transpose(
                                out=ptr[:kk, :],
                                in_=o1[:, bi * HC + k * P:bi * HC + k * P + kk],
                                identity=idn[:])
                            rT = pbb.tile([P, P], bf16, tag="rT")
                            nc.vector.tensor_copy(out=rT[:kk, :], in_=ptr[:kk, :])
                            nc.tensor.matmul(out=ph2[:], lhsT=rT[:kk, :],
                                             rhs=w2s[k][:kk, :],
                                             start=(k == 0), stop=(k == NCK - 1))
                        nc.vector.tensor_copy(out=h2s8[:, bi * R2:(bi + 1) * R2],
                                              in_=ph2[:])
                    # batched stores: h2pad always full blocks; h2loc clipped
                    nc.sync.dma_start(
                        out=bass.AP(h2pad, b0 * P * R2,
                                    [[R2, P], [P * R2, nblk], [1, R2]]),
                        in_=_sub(h2s8[:], 0, [[R2, nblk], [1, R2]]))
                    nfull = sum(1 for b in sb["blocks"] if (b + 1) * P <= NPC)
                    if nfull:
                        nc.sync.dma_start(
                            out=bass.AP(h2loc, b0 * P * R2,
                                        [[R2, P], [P * R2, nfull], [1, R2]]),
                            in_=_sub(h2s8[:], 0, [[R2, nfull], [1, R2]]))
                    for bi, b in enumerate(sb["blocks"]):
                        if bi < nfull:
                            continue
                        rows = NPC - b * P
                        if rows > 0:
                            nc.sync.dma_start(
                                out=h2loc[b * P:b * P + rows, :],
                                in_=h2s8[:rows, bi * R2:(bi + 1) * R2])

            # ---------------- AllGather + repack -----------------------------
            nc.gpsimd.collective_compute(
                "AllGather", mybir.AluOpType.bypass,
                replica_groups=[list(range(NC))],
                ins=[h2loc[:, :]], outs=[h2tab[:, :]])
            # repack [N, R2] -> 256B f32 rows [N, RB2]
            for r in range(NC):
                nc.sync.dma_start(
                    out=bass.AP(h2tabp, r * NPC * RB2, [[RB2, NPC], [1, R2]]),
                    in_=h2tab[r * NPC:(r + 1) * NPC, :])

            # ---------------- Phase C: L2 edge pass --------------------------
            with tc.tile_pool(name="pcg", bufs=2) as pcg, \
                 tc.tile_pool(name="pcb", bufs=2) as pcb, \
                 tc.tile_pool(name="psc", bufs=2, space="PSUM") as psc, \
                 tc.tile_pool(name="psd2", bufs=2, space="PSUM") as psd2:
                for sb in sb_meta:
                    base, S = sb["base"], sb["S"]
                    nblk = len(sb["blocks"])
                    b0 = sb["blocks"][0]
                    g2 = pcg.tile([P, S * RB2], f32, tag="g2")
                    ixs = pcg.tile([P, S * 8], i16, tag="ixs2")
                    nc.scalar.dma_start(out=ixs[:],
                                        in_=ihsrc_d[:, base * 8:(base + S) * 8])
                    for q in range(NCHUNK):
                        tb, segT = sb["segs"][q]
                        if segT == 0:
                            continue
                        gather_split(g2, tb - base, segT, RB2,
                                     h2tabp[q * CHB:min(q * CHB + nchrows[q], N), :],
                                     ixs)
                    adw2 = pcg.tile([P, 8], bf16, tag="adw2")
                    nc.gpsimd.dma_start(
                        out=adw2[:, :nblk],
                        in_=bass.AP(h2pad, b0 * P * R2 + CLS + 1,
                                    [[R2, P], [P * R2, nblk], [1, 1]]))
                    oh2 = pcb.tile([P, S * P], bf16, tag="oh2", bufs=1)
                    nc.vector.tensor_tensor(
                        out=oh2[:].rearrange("p (t q) -> p t q", t=S),
                        in0=_sub(iot[:], 0, [[0, S], [1, P]]),
                        in1=_sub(dlc[:], base, [[1, S], [0, P]]),
                        op=mybir.AluOpType.is_equal)
                    dlR = pcg.tile([P, S * P], bf16, tag="dlR2")
                    nc.scalar.dma_start(out=dlR[:],
                                        in_=dlocR_d[:, base * P:(base + S) * P])
                    oTs = pcb.tile([P, S * P], bf16, tag="oTs2", bufs=1)
                    nc.vector.tensor_scalar(
                        out=oTs[:], in0=dlR[:], scalar1=0.0, scalar2=None,
                        op0=mybir.AluOpType.is_equal)
                    pad2 = psd2.tile([P, S], f32, tag="pad2")
                    for bi, b in enumerate(sb["blocks"]):
                        for (tg, tt) in sb["runs"][b]:
                            for t in range(tt):
                                rel = tg - base + t
                                nc.tensor.matmul(
                                    out=pad2[:, rel:rel + 1],
                                    lhsT=oTs[:, rel * P:(rel + 1) * P],
                                    rhs=adw2[:, bi:bi + 1],
                                    start=True, stop=True,
                                    skip_group_check=True)
                    ex2 = pcb.tile([P, S], f32, tag="ex2")
                    nc.vector.tensor_tensor(
                        out=ex2[:],
                        in0=_sub(g2[:], CLS, [[RB2, S]]),
                        in1=pad2[:],
                        op=mybir.AluOpType.add)
                    tm2 = pcb.tile([P, S], f32, tag="tm2")
                    nc.vector.tensor_scalar_mul(out=tm2[:], in0=ex2[:], scalar1=NEG)
                    nc.vector.tensor_tensor(out=ex2[:], in0=ex2[:], in1=tm2[:],
                                            op=mybir.AluOpType.max)
                    nc.scalar.activation(out=ex2[:], in_=ex2[:],
                                         func=mybir.ActivationFunctionType.Exp)
                    m2 = pcb.tile([P, S * 4], bf16, tag="m2")
                    nc.vector.tensor_copy(out=_sub(m2[:], CLS, [[4, S]]), in_=ex2[:])
                    nc.vector.tensor_copy(out=_sub(m2[:], CLS + 1, [[4, S]]),
                                          in_=ex2[:])
                    nc.vector.tensor_tensor(
                        out=_sub(m2[:], 0, [[4, S], [1, CLS]]),
                        in0=_sub(g2[:], 0, [[RB2, S], [1, CLS]]),
                        in1=_sub(m2[:], CLS, [[4, S], [0, CLS]]),
                        op=mybir.AluOpType.mult)
                    v0 = pcb.tile([P, 4 * R2], f32, tag="v0", bufs=1)
                    for bi, b in enumerate(sb["blocks"]):
                        runs = sb["runs"][b]
                        ntile = sum(t for _, t in runs)
                        ps2 = psc.tile([P, 4], f32, tag="ps2")
                        ti = 0
                        for (tg, tt) in runs:
                            for t in range(tt):
                                rel = tg - base + t
                                nc.tensor.matmul(
                                    out=ps2[:],
                                    lhsT=oh2[:, rel * P:(rel + 1) * P],
                                    rhs=m2[:, rel * 4:(rel + 1) * 4],
                                    start=(ti == 0), stop=(ti == ntile - 1))
                                ti += 1
                        nc.vector.tensor_copy(out=v0[:, bi * R2:(bi + 1) * R2],
                                              in_=ps2[:])
                    # batched normalize + bias + log_softmax over the sb
                    dn2 = pcb.tile([P, 4], f32, tag="dn2")
                    nc.vector.tensor_scalar_max(
                        out=dn2[:, :nblk],
                        in0=_sub(v0[:], CLS, [[R2, nblk]]),
                        scalar1=1e-20)
                    rd2 = pcb.tile([P, 4], f32, tag="rd2")
                    nc.vector.reciprocal(out=rd2[:, :nblk], in_=dn2[:, :nblk])
                    vv = pcb.tile([P, 4 * CLS], f32, tag="vv")
                    nc.vector.tensor_tensor(
                        out=_sub(vv[:], 0, [[CLS, nblk], [1, CLS]]),
                        in0=_sub(v0[:], 0, [[R2, nblk], [1, CLS]]),
                        in1=_sub(rd2[:], 0, [[1, nblk], [0, CLS]]),
                        op=mybir.AluOpType.mult)
                    nc.vector.tensor_tensor(
                        out=_sub(vv[:], 0, [[CLS, nblk], [1, CLS]]),
                        in0=_sub(vv[:], 0, [[CLS, nblk], [1, CLS]]),
                        in1=_sub(b2s[:], 0, [[0, nblk], [1, CLS]]),
                        op=mybir.AluOpType.add)
                    mx = pcb.tile([P, 4], f32, tag="mx")
                    nc.vector.tensor_reduce(
                        out=_sub(mx[:], 0, [[1, nblk]]),
                        in_=_sub(vv[:], 0, [[CLS, nblk], [1, CLS]]),
                        axis=mybir.AxisListType.X,
                        op=mybir.AluOpType.max)
                    u = pcb.tile([P, 4 * CLS], f32, tag="u")
                    nc.vector.tensor_tensor(
                        out=_sub(u[:], 0, [[CLS, nblk], [1, CLS]]),
                        in0=_sub(vv[:], 0, [[CLS, nblk], [1, CLS]]),
                        in1=_sub(mx[:], 0, [[1, nblk], [0, CLS]]),
                        op=mybir.AluOpType.subtract)
                    nc.scalar.activation(out=u[:, :nblk * CLS],
                                         in_=u[:, :nblk * CLS],
                                         func=mybir.ActivationFunctionType.Exp)
                    sm = pcb.tile([P, 4], f32, tag="sm")
                    nc.vector.tensor_reduce(
                        out=_sub(sm[:], 0, [[1, nblk]]),
                        in_=_sub(u[:], 0, [[CLS, nblk], [1, CLS]]),
                        axis=mybir.AxisListType.X,
                        op=mybir.AluOpType.add)
                    ls = pcb.tile([P, 4], f32, tag="ls")
                    nc.scalar.activation(out=ls[:, :nblk], in_=sm[:, :nblk],
                                         func=mybir.ActivationFunctionType.Ln)
                    nc.vector.tensor_tensor(out=ls[:, :nblk], in0=ls[:, :nblk],
                                            in1=mx[:, :nblk],
                                            op=mybir.AluOpType.add)
                    res = pcb.tile([P, 4 * CLS], f32, tag="res")
                    nc.vector.tensor_tensor(
                        out=_sub(res[:], 0, [[CLS, nblk], [1, CLS]]),
                        in0=_sub(vv[:], 0, [[CLS, nblk], [1, CLS]]),
                        in1=_sub(ls[:], 0, [[1, nblk], [0, CLS]]),
                        op=mybir.AluOpType.subtract)
                    nfull = sum(1 for b in sb["blocks"] if (b + 1) * P <= NPC)
                    if nfull:
                        nc.sync.dma_start(
                            out=bass.AP(out_d, b0 * P * CLS,
                                        [[CLS, P], [P * CLS, nfull], [1, CLS]]),
                            in_=_sub(res[:], 0, [[CLS, nfull], [1, CLS]]))
                    for bi, b in enumerate(sb["blocks"]):
                        if bi < nfull:
                            continue
                        rows = NPC - b * P
                        if rows > 0:
                            nc.sync.dma_start(
                                out=out_d[b * P:b * P + rows, :],
                                in_=res[:rows, bi * CLS:(bi + 1) * CLS])
            _pa_stack.close()
    nc.finalize()
    return nc


def install_ntff_hook(so_path="/opt/axon/libaxon_pjrt.so"):
    import types
    import ctypes
    import contextlib
    import antenv

    if getattr(antenv, "axon_hooks", None) is not None:
        return
    lib = ctypes.CDLL(so_path)
    if not hasattr(lib, "axon_start_nrt_profile"):
        return
    lib.axon_start_nrt_profile.argtypes = [ctypes.POINTER(ctypes.c_int64),
                                           ctypes.c_size_t]
    lib.axon_start_nrt_profile.restype = ctypes.c_int64
    lib.axon_stop_nrt_profile.argtypes = [ctypes.c_char_p]
    lib.axon_stop_nrt_profile.restype = ctypes.c_int64

    @contextlib.contextmanager
    def _hook(output_dir, device_ids):
        import jax
        jax.devices()
        if device_ids:
            ids = (ctypes.c_int64 * len(device_ids))(*device_ids)
            rc = lib.axon_start_nrt_profile(ids, len(device_ids))
        else:
            rc = lib.axon_start_nrt_profile(None, 0)
        if rc != 0:
            raise RuntimeError(f"axon_start_nrt_profile rc={rc}")
        try:
            yield
        finally:
            n = lib.axon_stop_nrt_profile(str(output_dir).encode())
            print(f"ntff profile: {n} file(s) written to {output_dir}")

    mod = types.ModuleType("antenv.axon_hooks")
    _reg = [_hook]
    mod.set_axon_ntff_profile_hook = lambda h: _reg.__setitem__(0, h)
    mod.get_axon_ntff_profile_hook = lambda: _reg[0]
    sys.modules["antenv.axon_hooks"] = mod
    antenv.axon_hooks = mod


def run(inputs, cfg, trace=False, **kwargs):
    if trace:
        install_ntff_hook()
    in_maps, meta = prep(inputs, cfg)
    nc = build(meta)
    res = bass_utils.run_bass_kernel_spmd(
        nc, in_maps, core_ids=list(range(cfg["NC"])), trace=trace, **kwargs)
    out = np.concatenate([res.results[c]["out"] for c in range(cfg["NC"])], axis=0)
    return out, res


# ----------------------------------------------------------------------------
# harness entry point
# ----------------------------------------------------------------------------

_CFG = dict(N=100000, F=165, H=4, C=64, CLS=2, NC=8)


def kernel(**inputs):
    """Full (unsharded) inputs -> full [N, 2] float32 log-softmax output.

    Shards edges by destination-node range across the 8 NeuronCores,
    compiles and runs the Bass/Tile kernel via run_bass_kernel_spmd,
    and concatenates the per-core output slices.
    """
    out, _ = run(inputs, _CFG, trace=False)
    return np.ascontiguousarray(out.astype(np.float32))


# revision 62
# speedup vs baseline: 1.0770x; 1.0119x over previous
"""GAT 2-layer message-passing network on 8 TRN2 NeuronCores (Bass/Tile).

v3: dispatch/instruction-count optimized.

Strategy (dst-sharded):
 - Host: add self loops, sort edges by dst, shard dst-node ranges across cores.
   Each core owns nodes [c*NPC, (c+1)*NPC) and ALL edges into them.
 - Edge slots: per dst-block of 128 nodes, edges sub-grouped by src chunk
   (4 chunks of CH rows so int16 indices work), each (block,chunk) run padded
   to x128 slots = tiles. Superblocks of SBG blocks share gather calls.
 - Phase A (replicated): full feature table htab[n] = [h|a_src|pad] bf16
   [Np, 384] (768B rows for dma_gather), + local stats table sloc
   [NPCp, H] bf16 (a_dst of the core's own nodes), batched 8 tiles per DMA.
 - Phase B (L1): per sb: dma_gather htab rows by src (4 chunk calls);
   oT one-hot ([dst, slot]) via is_equal against the host-replicated
   dlocR table (no PE broadcast); per-tile matmuls oT x adw -> per-edge
   a_dst; ex = exp(lrelu(asrc+adst)) batched per sb; msg in-place in the
   gather buffer; one-hot oh from dloc2d vs iota; per-block PSUM matmul
   accumulation; per-sb batched normalize + b1 + relu; h2aug = relu @ W2aug
   via PE transpose; batched h2loc (AG input) + h2pad stores.
 - AllGather h2loc -> h2tab [N,4] f32; repack into h2tabp [Npp, 64] f32 rows.
 - Phase C (L2): same slots: gather h2tabp by src; same oT/oh structure;
   4-wide bf16 messages; one-hot matmuls; per-sb batched normalize, +b2,
   log_softmax -> out [NPC, 2] f32.
"""
import sys

if "/opt/trn_rl_repo" not in sys.path:
    sys.path.insert(0, "/opt/trn_rl_repo")

import math
import numpy as np
import ml_dtypes

import concourse.bass as bass
import concourse.bacc as bacc
import concourse.mybir as mybir
import concourse.tile as tile
from concourse import bass_utils

P = 128
NEG = 0.2
NCHUNK = 4
NQUEUE = 4
BAT = 8                      # phase-A tiles per DMA batch

# Tile's DMA sem-lane assignment round-robins over all DMAs of a DGE class,
# which breaks the per-lane FIFO assumption when DMAs run on multiple HW
# rings (out-of-order completion across rings under one counting sem):
#  - SWDGE (Pool) on multiple queues -> lane == queue_num.
#  - HWDGE from both SP (sync) and ACT (scalar) rings -> SP lanes 0-3,
#    ACT lanes 4-7 (per-engine round-robin).
from concourse import tile_sem_assignment as _tsa  # noqa: E402

if not getattr(_tsa.TileClockTick, "_qaware_patched", False):
    _orig_assign_tick = _tsa.TileClockTick._assign_tick

    def _qaware_assign_tick(self, inst):
        if isinstance(inst, _tsa.DMAInst):
            if inst.engine == mybir.EngineType.Pool:
                self.next_sw_dma_idx = getattr(inst, "queue_num", 0) or 0
            elif inst.engine in (mybir.EngineType.SP,
                                 mybir.EngineType.Activation):
                if not hasattr(self, "_hw_rr"):
                    self._hw_rr = [0, 0]
                w = 1 if inst.engine == mybir.EngineType.Activation else 0
                self.next_hw_dma_idx = w * 4 + (self._hw_rr[w] % 4)
                self._hw_rr[w] += 1
        return _orig_assign_tick(self, inst)

    _tsa.TileClockTick._assign_tick = _qaware_assign_tick
    _tsa.TileClockTick._qaware_patched = True


def _wrap16(flat):
    """[n] -> [128, n//16] wrapped in 16 partitions, replicated x8."""
    w = flat.reshape(-1, 16).T
    return np.tile(w, (8, 1))


# ----------------------------------------------------------------------------
# host-side data prep
# ----------------------------------------------------------------------------

def prep(inputs, cfg):
    N, F, H, C, CLS, NC = cfg["N"], cfg["F"], cfg["H"], cfg["C"], cfg["CLS"], cfg["NC"]
    SBG = cfg.get("SBG", 4)
    x = np.asarray(inputs["x"], np.float32)
    ei = np.asarray(inputs["edge_index"])
    W1 = np.asarray(inputs["W1"], np.float32)
    as1 = np.asarray(inputs["att_src1"], np.float32)
    ad1 = np.asarray(inputs["att_dst1"], np.float32)
    b1 = np.asarray(inputs["b1"], np.float32)
    W2 = np.asarray(inputs["W2"], np.float32)
    as2 = np.asarray(inputs["att_src2"], np.float32)
    ad2 = np.asarray(inputs["att_dst2"], np.float32)
    b2 = np.asarray(inputs["b2"], np.float32)

    HC = H * C
    R1 = HC + 2 * H                      # live row payload [h | asrc | adst]
    RG = 128 * math.ceil(R1 / 128)       # htab gather row elems (bf16, 256B mult)
    NPC = N // NC
    NB = math.ceil(NPC / P)
    NPCp = NB * P                        # padded local rows
    NT = (N + P - 1) // P
    Np = NT * P
    # chunk base, tile-aligned so phase-A stores land in per-chunk ranges
    # (lets chunk-q gathers start before the whole table is written)
    CHB = math.ceil(N / NCHUNK / P) * P
    assert Np - (NCHUNK - 1) * CHB < 32768  # last-chunk slice
    assert CHB < 32768 and NPCp < 32768

    # ---- weights / constants -------------------------------------------------
    W1r = W1.reshape(F, H, C)
    Wsrc = np.einsum("fhc,hc->fh", W1r, as1)
    Wdst = np.einsum("fhc,hc->fh", W1r, ad1)
    W1aug = np.concatenate([W1, Wsrc, Wdst], axis=1)          # [F, R1]
    Wsrc2 = W2 @ as2.reshape(CLS, 1)
    Wdst2 = W2 @ ad2.reshape(CLS, 1)
    W2aug = np.concatenate([W2, Wsrc2, Wdst2], axis=1)        # [HC, 4]

    bf16 = ml_dtypes.bfloat16
    xT = np.zeros((F, Np), dtype=bf16)
    xT[:, :N] = x.T.astype(bf16)
    W1aug_b = W1aug.astype(bf16)
    W2aug_b = W2aug.astype(bf16)
    b1rep = np.tile(b1[None, :], (P, 1)).astype(bf16)
    b2rep = np.tile(b2[None, :], (P, 1)).astype(np.float32)
    iota = np.tile(np.arange(P, dtype=np.float32)[None, :], (P, 1)).astype(bf16)
    ident = np.eye(P, dtype=bf16)

    # ---- edges ---------------------------------------------------------------
    src_all = np.concatenate([ei[0], np.arange(N, dtype=ei.dtype)]).astype(np.int64)
    dst_all = np.concatenate([ei[1], np.arange(N, dtype=ei.dtype)]).astype(np.int64)
    order = np.argsort(dst_all, kind="stable")
    src_s = src_all[order]
    dst_s = dst_all[order]
    chunk_s = src_s // CHB

    cnts = np.zeros((NC, NB, NCHUNK), np.int64)
    for c in range(NC):
        for b in range(NB):
            base = c * NPC + b * P
            hi = min(base + P, (c + 1) * NPC)
            lo_i = np.searchsorted(dst_s, base)
            hi_i = np.searchsorted(dst_s, hi)
            ch = chunk_s[lo_i:hi_i]
            for q in range(NCHUNK):
                cnts[c, b, q] = (ch == q).sum()
    Trun = np.ceil(cnts / P).astype(np.int64).max(axis=0)     # [NB, NCHUNK]

    # superblocks
    sblocks = [list(range(i, min(i + SBG, NB))) for i in range(0, NB, SBG)]
    # slot layout: per sb: for q: for b in sb: Trun[b,q] tiles
    sb_meta = []
    tile_base = 0
    for blist in sblocks:
        segs = []           # per q: (seg_tile_base_global, segT)
        runs = {b: [] for b in blist}   # block -> [(tile_global, T)]
        sb_base = tile_base
        for q in range(NCHUNK):
            segT = int(Trun[blist, q].sum())
            segs.append((tile_base, segT))
            tb = tile_base
            for b in blist:
                t = int(Trun[b, q])
                if t:
                    runs[b].append((tb, t))
                tb += t
            tile_base += segT
        sb_meta.append(dict(base=sb_base, S=tile_base - sb_base, segs=segs,
                            blocks=blist, runs=runs))
    Tsum = tile_base

    # per-core slot-value arrays
    ihsrc_w = np.zeros((NC, P, Tsum * 8), np.int16)
    dloc2d = np.full((NC, P, Tsum), 255.0, bf16)
    dlocR_a = np.zeros((NC, P, Tsum * P), bf16)
    for c in range(NC):
        ihsrc = np.zeros(Tsum * P, np.int16)
        dloc = np.full(Tsum * P, 255.0, np.float32)
        core_lo = np.searchsorted(dst_s, c * NPC)
        core_hi = np.searchsorted(dst_s, (c + 1) * NPC)
        cs, cd, cq = (src_s[core_lo:core_hi], dst_s[core_lo:core_hi],
                      chunk_s[core_lo:core_hi])
        # edges sorted by (dst, chunk); regroup per (block, chunk)
        for sb in sb_meta:
            for q in range(NCHUNK):
                for b in sb["blocks"]:
                    t = int(Trun[b, q])
                    if t == 0:
                        continue
                    # this block+chunk's edges (mask within the dst range)
                    base = c * NPC + b * P
                    hi = min(base + P, (c + 1) * NPC)
                    seg = slice(np.searchsorted(cd, base), np.searchsorted(cd, hi))
                    m = cq[seg] == q
                    es, ed = cs[seg][m], cd[seg][m]
                    n = len(es)
                    assert n <= t * P
                    # locate this run's global tile index (runs are in q order)
                    tg = None
                    for (tgi, tti) in sb["runs"][b]:
                        s0, sT = sb["segs"][q]
                        if s0 <= tgi < s0 + sT:
                            tg = tgi
                            break
                    assert tg is not None
                    s0 = tg * P
                    ihsrc[s0:s0 + n] = (es - q * CHB).astype(np.int16)
                    dloc[s0:s0 + n] = (ed - (c * NPC + b * P)).astype(np.float32)
        ihsrc_w[c] = _wrap16(ihsrc)
        dloc2d[c] = dloc.reshape(Tsum, P).T.astype(bf16)
        # dloc[s] - p per partition: oT = (dlz == 0) via fast tensor_scalar
        dlocR_a[c] = (dloc[None, :]
                      - np.arange(P, dtype=np.float32)[:, None]).astype(bf16)

    shared = {
        "xT": xT, "W1aug": W1aug_b, "W2aug": W2aug_b, "b1rep": b1rep,
        "b2rep": b2rep, "iota": iota, "ident": ident,
    }
    in_maps = []
    for c in range(NC):
        m = dict(shared)
        xl = np.zeros((F, NPCp), dtype=bf16)
        xl[:, :NPC] = xT[:, c * NPC:c * NPC + NPC]
        m["xTloc"] = xl
        m["ihsrc"] = ihsrc_w[c]
        m["dloc2d"] = dloc2d[c]
        m["dlocR"] = dlocR_a[c]
        in_maps.append(m)

    meta = dict(cfg, R1=R1, RG=RG, HC=HC, NPC=NPC, NPCp=NPCp, NB=NB, NT=NT,
                Np=Np, CHB=CHB, Tsum=Tsum, sb_meta=sb_meta, SBG=SBG)
    return in_maps, meta


# ----------------------------------------------------------------------------
# device program
# ----------------------------------------------------------------------------

def _sub(ap, elem_off, dims):
    return bass.AP(ap.tensor, ap.offset + elem_off, [ap.ap[0], *list(dims)])


def build(meta, nc=None):
    N, F, H, C, CLS = meta["N"], meta["F"], meta["H"], meta["C"], meta["CLS"]
    NC, R1, RG, HC = meta["NC"], meta["R1"], meta["RG"], meta["HC"]
    NPC, NPCp, NB, NT, Np = (meta["NPC"], meta["NPCp"], meta["NB"], meta["NT"],
                             meta["Np"])
    CHB, Tsum = meta["CHB"], meta["Tsum"]
    sb_meta = meta["sb_meta"]
    R2 = CLS + 2
    RB2 = 64                           # f32 row elems for L2 gather tables

    f32, bf16, i16 = mybir.dt.float32, mybir.dt.bfloat16, mybir.dt.int16

    if nc is None:
        nc = bacc.Bacc("TRN2", target_bir_lowering=False, debug=False,
                       num_devices=NC, num_swdge_queues=NQUEUE)

    MAXT = 7                 # tiles per dma_gather call (<=896 descs, carveout 1024)
    qrr = [0]

    def gather_split(out_tile, rel, segT, elem, table, ix_tile):
        """Split a segment gather into <=MAXT-tile calls, round-robin queues."""
        done = 0
        while done < segT:
            tt = min(MAXT, segT - done)
            r = rel + done
            nc.gpsimd.dma_gather(
                bass.AP(out_tile[:].tensor, out_tile[:].offset + r * elem,
                        [out_tile[:].ap[0], [elem, tt], [1, elem]]),
                table,
                ix_tile[:, r * 8:(r + tt) * 8],
                tt * P, tt * P, elem,
                queue_num=qrr[0] % NQUEUE,
            )
            qrr[0] += 1
            done += tt

    xT_d = nc.dram_tensor("xT", [F, Np], bf16, kind="ExternalInput")
    xTl_d = nc.dram_tensor("xTloc", [F, NPCp], bf16, kind="ExternalInput")
    W1aug_d = nc.dram_tensor("W1aug", [F, R1], bf16, kind="ExternalInput")
    W2aug_d = nc.dram_tensor("W2aug", [HC, R2], bf16, kind="ExternalInput")
    b1rep_d = nc.dram_tensor("b1rep", [P, HC], bf16, kind="ExternalInput")
    b2rep_d = nc.dram_tensor("b2rep", [P, CLS], f32, kind="ExternalInput")
    iota_d = nc.dram_tensor("iota", [P, P], bf16, kind="ExternalInput")
    ident_d = nc.dram_tensor("ident", [P, P], bf16, kind="ExternalInput")
    ihsrc_d = nc.dram_tensor("ihsrc", [P, Tsum * 8], i16, kind="ExternalInput")
    dloc_d = nc.dram_tensor("dloc2d", [P, Tsum], bf16, kind="ExternalInput")
    dlocR_d = nc.dram_tensor("dlocR", [P, Tsum * P], bf16, kind="ExternalInput")
    out_d = nc.dram_tensor("out", [NPC, CLS], f32, kind="ExternalOutput")

    # per-chunk h tables (separate tensors so chunk-q gathers only depend on
    # chunk-q phase-A stores)
    nchrows = [CHB] * (NCHUNK - 1) + [Np - (NCHUNK - 1) * CHB]
    htabq = [nc.dram_tensor(f"htab{q}", [nchrows[q], RG], bf16, kind="Internal")
             for q in range(NCHUNK)]
    sloc = nc.dram_tensor("sloc", [NPCp, H], bf16, kind="Internal")
    h2loc = nc.dram_tensor("h2loc", [NPC, R2], f32, kind="Internal")
    h2pad = nc.dram_tensor("h2pad", [NPCp, R2], f32, kind="Internal")
    h2tab = nc.dram_tensor("h2tab", [N, R2], f32, kind="Internal",
                           addr_space="Shared" if NC > 4 else "Local")
    h2tabp = nc.dram_tensor("h2tabp", [N, RB2], f32, kind="Internal")

    FA = min(P, F)
    FB = F - FA
    NCK = (HC + P - 1) // P

    with tile.TileContext(nc) as tc:
        with tc.tile_pool(name="const", bufs=1) as cp:
            w1a = cp.tile([FA, R1], bf16)
            nc.sync.dma_start(out=w1a[:], in_=W1aug_d[0:FA, :])
            if FB:
                w1b = cp.tile([FB, R1], bf16)
                nc.sync.dma_start(out=w1b[:], in_=W1aug_d[FA:F, :])
            w2s = []
            for k in range(NCK):
                kk = min(P, HC - k * P)
                w2k = cp.tile([kk, R2], bf16, name=f"w2k{k}")
                nc.sync.dma_start(out=w2k[:], in_=W2aug_d[k * P:k * P + kk, :])
                w2s.append(w2k)
            b1s = cp.tile([P, HC], bf16)
            nc.sync.dma_start(out=b1s[:], in_=b1rep_d[:, :])
            b2s = cp.tile([P, CLS], f32)
            nc.sync.dma_start(out=b2s[:], in_=b2rep_d[:, :])
            iot = cp.tile([P, P], bf16)
            nc.sync.dma_start(out=iot[:], in_=iota_d[:, :])
            idn = cp.tile([P, P], bf16)
            nc.sync.dma_start(out=idn[:], in_=ident_d[:, :])
            dlc = cp.tile([P, Tsum], bf16)
            nc.sync.dma_start(out=dlc[:], in_=dloc_d[:, :])

            # ---------------- Phase A: feature tables ------------------------
            # local a_dst stats FIRST so phase-B adw loads unblock early.
            # pa (SBUF) stays open through B/C so phase-B tiles don't reuse
            # its addresses (address-reuse WAR would chain gathers behind
            # the whole of phase A); only the PSUM pool closes.
            import contextlib
            _pa_stack = contextlib.ExitStack()
            pa = _pa_stack.enter_context(tc.tile_pool(name="pa", bufs=2))
            with tc.tile_pool(name="psa", bufs=4, space="PSUM") as psa:
                for g8 in range(0, NPCp // P, BAT):
                    nb8 = min(BAT, NPCp // P - g8)
                    xa = pa.tile([FA, BAT * P], bf16, tag="xla")
                    nc.sync.dma_start(out=xa[:, :nb8 * P],
                                      in_=xTl_d[0:FA, g8 * P:(g8 + nb8) * P])
                    if FB:
                        xb = pa.tile([FB, BAT * P], bf16, tag="xlb")
                        nc.sync.dma_start(out=xb[:, :nb8 * P],
                                          in_=xTl_d[FA:F, g8 * P:(g8 + nb8) * P])
                    ss8 = pa.tile([P, BAT * H], bf16, tag="ss8")
                    for k in range(nb8):
                        ps = psa.tile([P, H], f32, tag="pss")
                        nc.tensor.matmul(out=ps[:], lhsT=xa[:, k * P:(k + 1) * P],
                                         rhs=w1a[:, HC + H:HC + 2 * H],
                                         start=True, stop=(FB == 0))
                        if FB:
                            nc.tensor.matmul(out=ps[:],
                                             lhsT=xb[:, k * P:(k + 1) * P],
                                             rhs=w1b[:, HC + H:HC + 2 * H],
                                             start=False, stop=True)
                        nc.vector.tensor_copy(out=ss8[:, k * H:(k + 1) * H],
                                              in_=ps[:])
                    nc.sync.dma_start(
                        out=bass.AP(sloc, g8 * P * H,
                                    [[H, P], [P * H, nb8], [1, H]]),
                        in_=_sub(ss8[:], 0, [[H, nb8], [1, H]]))
                for q in range(NCHUNK):
                    qt0 = q * CHB // P
                    qnt = min(NT, (q * CHB + nchrows[q]) // P) - qt0
                    for g8 in range(qt0, qt0 + qnt, BAT):
                        nb8 = min(BAT, qt0 + qnt - g8)
                        xa = pa.tile([FA, BAT * P], bf16, tag="xa")
                        nc.sync.dma_start(out=xa[:, :nb8 * P],
                                          in_=xT_d[0:FA, g8 * P:(g8 + nb8) * P])
                        if FB:
                            xb = pa.tile([FB, BAT * P], bf16, tag="xb")
                            nc.sync.dma_start(
                                out=xb[:, :nb8 * P],
                                in_=xT_d[FA:F, g8 * P:(g8 + nb8) * P])
                        hs8 = pa.tile([P, BAT * R1], bf16, tag="hs8")
                        for k in range(nb8):
                            ph = psa.tile([P, R1], f32, tag="ph")
                            nc.tensor.matmul(out=ph[:],
                                             lhsT=xa[:, k * P:(k + 1) * P],
                                             rhs=w1a[:], start=True,
                                             stop=(FB == 0))
                            if FB:
                                nc.tensor.matmul(out=ph[:],
                                                 lhsT=xb[:, k * P:(k + 1) * P],
                                                 rhs=w1b[:], start=False,
                                                 stop=True)
                            nc.vector.tensor_copy(
                                out=hs8[:, k * R1:(k + 1) * R1], in_=ph[:])
                        nc.sync.dma_start(
                            out=bass.AP(htabq[q], (g8 - qt0) * P * RG,
                                        [[RG, P], [P * RG, nb8], [1, R1]]),
                            in_=_sub(hs8[:], 0, [[R1, nb8], [1, R1]]))

            # ---------------- Phase B: L1 edge pass --------------------------
            with tc.tile_pool(name="pbg", bufs=2) as pbg, \
                 tc.tile_pool(name="pbb", bufs=2) as pbb, \
                 tc.tile_pool(name="psb", bufs=2, space="PSUM") as psb, \
                 tc.tile_pool(name="pst", bufs=2, space="PSUM") as pst, \
                 tc.tile_pool(name="psh", bufs=2, space="PSUM") as psh, \
                 tc.tile_pool(name="psa2", bufs=2, space="PSUM") as psa2:
                for sb in sb_meta:
                    base, S = sb["base"], sb["S"]
                    nblk = len(sb["blocks"])
                    b0 = sb["blocks"][0]
                    g = pbg.tile([P, S * RG], bf16, tag="g")
                    ixs = pbg.tile([P, S * 8], i16, tag="ixs")
                    nc.scalar.dma_start(out=ixs[:],
                                        in_=ihsrc_d[:, base * 8:(base + S) * 8])
                    for q in range(NCHUNK):
                        tb, segT = sb["segs"][q]
                        if segT == 0:
                            continue
                        gather_split(g, tb - base, segT, RG, htabq[q][:, :], ixs)
                    # a_dst window for the sb's blocks  [P, nblk*H] bf16
                    adw = pbg.tile([P, 8 * H], bf16, tag="adw")
                    nc.scalar.dma_start(
                        out=adw[:, :nblk * H],
                        in_=bass.AP(sloc, b0 * P * H,
                                    [[H, P], [P * H, nblk], [1, H]]))
                    # one-hot for all slots  [P, S*P] bf16 (two halves)
                    oh = pbb.tile([P, S * P], bf16, tag="oh", bufs=1)
                    OH2 = (S + 1) // 2
                    for z0 in range(0, S, OH2):
                        nz = min(OH2, S - z0)
                        nc.vector.tensor_tensor(
                            out=_sub(oh[:], z0 * P, [[P, nz], [1, P]]),
                            in0=_sub(iot[:], 0, [[0, nz], [1, P]]),
                            in1=_sub(dlc[:], base + z0, [[1, nz], [0, P]]),
                            op=mybir.AluOpType.is_equal)
                    # O_T: [d, slot] one-hot via host (dloc - p) table
                    dlR = pbg.tile([P, S * P], bf16, tag="dlR")
                    nc.scalar.dma_start(out=dlR[:],
                                        in_=dlocR_d[:, base * P:(base + S) * P])
                    oTs = pbb.tile([P, S * P], bf16, tag="oTs", bufs=1)
                    nc.vector.tensor_scalar(
                        out=oTs[:], in0=dlR[:], scalar1=0.0, scalar2=None,
                        op0=mybir.AluOpType.is_equal)
                    # per-edge a_dst: oT x adw matmuls -> PSUM [P, S*H]
                    pad = psa2.tile([P, S * H], f32, tag="pad")
                    for bi, b in enumerate(sb["blocks"]):
                        for (tg, tt) in sb["runs"][b]:
                            for t in range(tt):
                                rel = tg - base + t
                                nc.tensor.matmul(
                                    out=pad[:, rel * H:(rel + 1) * H],
                                    lhsT=oTs[:, rel * P:(rel + 1) * P],
                                    rhs=adw[:, bi * H:(bi + 1) * H],
                                    start=True, stop=True,
                                    skip_group_check=True)
                    # ex = exp(lrelu(asrc+adst)), msg multiply and denom
                    # copies, all SEGMENT-aligned: each chunk-segment's chain
                    # depends only on its own gather, so segment-0 compute
                    # overlaps segments 1-3 still draining.
                    ex = pbb.tile([P, S * H], f32, tag="ex", bufs=1)
                    tmp = pbb.tile([P, S * H], f32, tag="tmp", bufs=1)
                    exb = pbb.tile([P, S * H], bf16, tag="exb", bufs=1)
                    for qi in range(NCHUNK):
                        tbq, segT = sb["segs"][qi]
                        if segT == 0:
                            continue
                        s0 = tbq - base
                        ns = segT
                        exv = ex[:, s0 * H:(s0 + ns) * H]
                        tmv = tmp[:, s0 * H:(s0 + ns) * H]
                        nc.vector.tensor_tensor(
                            out=_sub(ex[:], s0 * H, [[H, ns], [1, H]]),
                            in0=_sub(g[:], s0 * RG + HC, [[RG, ns], [1, H]]),
                            in1=_sub(pad[:], s0 * H, [[H, ns], [1, H]]),
                            op=mybir.AluOpType.add)
                        nc.vector.tensor_scalar_mul(out=tmv, in0=exv,
                                                    scalar1=NEG)
                        nc.vector.tensor_tensor(out=exv, in0=exv, in1=tmv,
                                                op=mybir.AluOpType.max)
                        nc.scalar.activation(
                            out=exv, in_=exv,
                            func=mybir.ActivationFunctionType.Exp)
                        nc.vector.tensor_copy(
                            out=exb[:, s0 * H:(s0 + ns) * H], in_=exv)
                        nc.vector.tensor_tensor(
                            out=_sub(g[:], s0 * RG, [[RG, ns], [C, H], [1, C]]),
                            in0=_sub(g[:], s0 * RG, [[RG, ns], [C, H], [1, C]]),
                            in1=_sub(exb[:], s0 * H, [[H, ns], [1, H], [0, C]]),
                            op=mybir.AluOpType.mult)
                        nc.vector.tensor_copy(
                            out=_sub(g[:], s0 * RG + HC, [[RG, ns], [1, H]]),
                            in_=_sub(exb[:], s0 * H, [[H, ns], [1, H]]))
                        nc.vector.tensor_copy(
                            out=_sub(g[:], s0 * RG + HC + H, [[RG, ns], [1, H]]),
                            in_=_sub(exb[:], s0 * H, [[H, ns], [1, H]]))
                    # per-block accumulation
                    po = pbb.tile([P, 4 * R1], f32, tag="po", bufs=1)
                    for bi, b in enumerate(sb["blocks"]):
                        runs = sb["runs"][b]
                        ntile = sum(t for _, t in runs)
                        pso = psb.tile([P, R1], f32, tag="pso")
                        ti = 0
                        for (tg, tt) in runs:
                            for t in range(tt):
                                rel = tg - base + t
                                nc.tensor.matmul(
                                    out=pso[:],
                                    lhsT=oh[:, rel * P:(rel + 1) * P],
                                    rhs=g[:, rel * RG:rel * RG + R1],
                                    start=(ti == 0), stop=(ti == ntile - 1))
                                ti += 1
                        nc.vector.tensor_copy(out=po[:, bi * R1:(bi + 1) * R1],
                                              in_=pso[:])
                    # batched normalize + bias + relu over the sb's blocks
                    den = pbb.tile([P, 4 * H], f32, tag="den")
                    nc.vector.tensor_scalar_max(
                        out=den[:, :nblk * H],
                        in0=_sub(po[:], HC, [[R1, nblk], [1, H]]),
                        scalar1=1e-20)
                    rde = pbb.tile([P, 4 * H], f32, tag="rde")
                    nc.vector.reciprocal(out=rde[:, :nblk * H],
                                         in_=den[:, :nblk * H])
                    o1 = pbb.tile([P, 4 * HC], bf16, tag="o1")
                    nc.vector.tensor_tensor(
                        out=_sub(o1[:], 0, [[HC, nblk], [C, H], [1, C]]),
                        in0=_sub(po[:], 0, [[R1, nblk], [C, H], [1, C]]),
                        in1=_sub(rde[:], 0, [[H, nblk], [1, H], [0, C]]),
                        op=mybir.AluOpType.mult)
                    nc.vector.tensor_tensor(
                        out=_sub(o1[:], 0, [[HC, nblk], [1, HC]]),
                        in0=_sub(o1[:], 0, [[HC, nblk], [1, HC]]),
                        in1=_sub(b1s[:], 0, [[0, nblk], [1, HC]]),
                        op=mybir.AluOpType.add)
                    nc.scalar.activation(out=o1[:, :nblk * HC],
                                         in_=o1[:, :nblk * HC],
                                         func=mybir.ActivationFunctionType.Relu)
                    # second layer projection per block (PE transpose path)
                    h2s8 = pbb.tile([P, 4 * R2], f32, tag="h2s8", bufs=1)
                    for bi, b in enumerate(sb["blocks"]):
                        ph2 = psh.tile([P, R2], f32, tag="ph2")
                        for k in range(NCK):
                            kk = min(P, HC - k * P)
                            ptr = pst.tile([P, P], bf16, tag="ptr")
                            nc.tensor.transpose(
                                out=ptr[:kk, :],
                                in_=o1[:, bi * HC + k * P:bi * HC + k * P + kk],
                                identity=idn[:])
                            rT = pbb.tile([P, P], bf16, tag="rT")
                            nc.vector.tensor_copy(out=rT[:kk, :], in_=ptr[:kk, :])
                            nc.tensor.matmul(out=ph2[:], lhsT=rT[:kk, :],
                                             rhs=w2s[k][:kk, :],
                                             start=(k == 0), stop=(k == NCK - 1))
                        nc.vector.tensor_copy(out=h2s8[:, bi * R2:(bi + 1) * R2],
                                              in_=ph2[:])
                    # batched stores: h2pad always full blocks; h2loc clipped
                    nc.sync.dma_start(
                        out=bass.AP(h2pad, b0 * P * R2,
                                    [[R2, P], [P * R2, nblk], [1, R2]]),
                        in_=_sub(h2s8[:], 0, [[R2, nblk], [1, R2]]))
                    nfull = sum(1 for b in sb["blocks"] if (b + 1) * P <= NPC)
                    if nfull:
                        nc.sync.dma_start(
                            out=bass.AP(h2loc, b0 * P * R2,
                                        [[R2, P], [P * R2, nfull], [1, R2]]),
                            in_=_sub(h2s8[:], 0, [[R2, nfull], [1, R2]]))
                    for bi, b in enumerate(sb["blocks"]):
                        if bi < nfull:
                            continue
                        rows = NPC - b * P
                        if rows > 0:
                            nc.sync.dma_start(
                                out=h2loc[b * P:b * P + rows, :],
                                in_=h2s8[:rows, bi * R2:(bi + 1) * R2])

            # ---------------- AllGather + repack -----------------------------
            nc.gpsimd.collective_compute(
                "AllGather", mybir.AluOpType.bypass,
                replica_groups=[list(range(NC))],
                ins=[h2loc[:, :]], outs=[h2tab[:, :]])
            # repack [N, R2] -> 256B f32 rows [N, RB2]
            for r in range(NC):
                nc.sync.dma_start(
                    out=bass.AP(h2tabp, r * NPC * RB2, [[RB2, NPC], [1, R2]]),
                    in_=h2tab[r * NPC:(r + 1) * NPC, :])

            # ---------------- Phase C: L2 edge pass --------------------------
            with tc.tile_pool(name="pcg", bufs=2) as pcg, \
                 tc.tile_pool(name="pcb", bufs=2) as pcb, \
                 tc.tile_pool(name="psc", bufs=2, space="PSUM") as psc, \
                 tc.tile_pool(name="psd2", bufs=2, space="PSUM") as psd2:
                for sb in sb_meta:
                    base, S = sb["base"], sb["S"]
                    nblk = len(sb["blocks"])
                    b0 = sb["blocks"][0]
                    g2 = pcg.tile([P, S * RB2], f32, tag="g2")
                    ixs = pcg.tile([P, S * 8], i16, tag="ixs2")
                    nc.scalar.dma_start(out=ixs[:],
                                        in_=ihsrc_d[:, base * 8:(base + S) * 8])
                    for q in range(NCHUNK):
                        tb, segT = sb["segs"][q]
                        if segT == 0:
                            continue
                        gather_split(g2, tb - base, segT, RB2,
                                     h2tabp[q * CHB:min(q * CHB + nchrows[q], N), :],
                                     ixs)
                    adw2 = pcg.tile([P, 8], bf16, tag="adw2")
                    nc.gpsimd.dma_start(
                        out=adw2[:, :nblk],
                        in_=bass.AP(h2pad, b0 * P * R2 + CLS + 1,
                                    [[R2, P], [P * R2, nblk], [1, 1]]))
                    oh2 = pcb.tile([P, S * P], bf16, tag="oh2", bufs=1)
                    nc.vector.tensor_tensor(
                        out=oh2[:].rearrange("p (t q) -> p t q", t=S),
                        in0=_sub(iot[:], 0, [[0, S], [1, P]]),
                        in1=_sub(dlc[:], base, [[1, S], [0, P]]),
                        op=mybir.AluOpType.is_equal)
                    dlR = pcg.tile([P, S * P], bf16, tag="dlR2")
                    nc.scalar.dma_start(out=dlR[:],
                                        in_=dlocR_d[:, base * P:(base + S) * P])
                    oTs = pcb.tile([P, S * P], bf16, tag="oTs2", bufs=1)
                    nc.vector.tensor_scalar(
                        out=oTs[:], in0=dlR[:], scalar1=0.0, scalar2=None,
                        op0=mybir.AluOpType.is_equal)
                    pad2 = psd2.tile([P, S], f32, tag="pad2")
                    for bi, b in enumerate(sb["blocks"]):
                        for (tg, tt) in sb["runs"][b]:
                            for t in range(tt):
                                rel = tg - base + t
                                nc.tensor.matmul(
                                    out=pad2[:, rel:rel + 1],
                                    lhsT=oTs[:, rel * P:(rel + 1) * P],
                                    rhs=adw2[:, bi:bi + 1],
                                    start=True, stop=True,
                                    skip_group_check=True)
                    # ex2 / m2 chains segment-aligned (as in L1): each
                    # chunk-segment depends only on its own gather drain
                    ex2 = pcb.tile([P, S], f32, tag="ex2")
                    tm2 = pcb.tile([P, S], f32, tag="tm2")
                    m2 = pcb.tile([P, S * 4], bf16, tag="m2")
                    for qi in range(NCHUNK):
                        tbq, segT = sb["segs"][qi]
                        if segT == 0:
                            continue
                        s0 = tbq - base
                        ns = segT
                        e2v = ex2[:, s0:s0 + ns]
                        t2v = tm2[:, s0:s0 + ns]
                        nc.vector.tensor_tensor(
                            out=e2v,
                            in0=_sub(g2[:], s0 * RB2 + CLS, [[RB2, ns]]),
                            in1=pad2[:, s0:s0 + ns],
                            op=mybir.AluOpType.add)
                        nc.vector.tensor_scalar_mul(out=t2v, in0=e2v,
                                                    scalar1=NEG)
                        nc.vector.tensor_tensor(out=e2v, in0=e2v, in1=t2v,
                                                op=mybir.AluOpType.max)
                        nc.scalar.activation(
                            out=e2v, in_=e2v,
                            func=mybir.ActivationFunctionType.Exp)
                        nc.vector.tensor_copy(
                            out=_sub(m2[:], s0 * 4 + CLS, [[4, ns]]), in_=e2v)
                        nc.vector.tensor_copy(
                            out=_sub(m2[:], s0 * 4 + CLS + 1, [[4, ns]]),
                            in_=e2v)
                        nc.vector.tensor_tensor(
                            out=_sub(m2[:], s0 * 4, [[4, ns], [1, CLS]]),
                            in0=_sub(g2[:], s0 * RB2, [[RB2, ns], [1, CLS]]),
                            in1=_sub(m2[:], s0 * 4 + CLS, [[4, ns], [0, CLS]]),
                            op=mybir.AluOpType.mult)
                    v0 = pcb.tile([P, 4 * R2], f32, tag="v0", bufs=1)
                    for bi, b in enumerate(sb["blocks"]):
                        runs = sb["runs"][b]
                        ntile = sum(t for _, t in runs)
                        ps2 = psc.tile([P, 4], f32, tag="ps2")
                        ti = 0
                        for (tg, tt) in runs:
                            for t in range(tt):
                                rel = tg - base + t
                                nc.tensor.matmul(
                                    out=ps2[:],
                                    lhsT=oh2[:, rel * P:(rel + 1) * P],
                                    rhs=m2[:, rel * 4:(rel + 1) * 4],
                                    start=(ti == 0), stop=(ti == ntile - 1))
                                ti += 1
                        nc.vector.tensor_copy(out=v0[:, bi * R2:(bi + 1) * R2],
                                              in_=ps2[:])
                    # batched normalize + bias + log_softmax over the sb
                    dn2 = pcb.tile([P, 4], f32, tag="dn2")
                    nc.vector.tensor_scalar_max(
                        out=dn2[:, :nblk],
                        in0=_sub(v0[:], CLS, [[R2, nblk]]),
                        scalar1=1e-20)
                    rd2 = pcb.tile([P, 4], f32, tag="rd2")
                    nc.vector.reciprocal(out=rd2[:, :nblk], in_=dn2[:, :nblk])
                    vv = pcb.tile([P, 4 * CLS], f32, tag="vv")
                    nc.vector.tensor_tensor(
                        out=_sub(vv[:], 0, [[CLS, nblk], [1, CLS]]),
                        in0=_sub(v0[:], 0, [[R2, nblk], [1, CLS]]),
                        in1=_sub(rd2[:], 0, [[1, nblk], [0, CLS]]),
                        op=mybir.AluOpType.mult)
                    nc.vector.tensor_tensor(
                        out=_sub(vv[:], 0, [[CLS, nblk], [1, CLS]]),
                        in0=_sub(vv[:], 0, [[CLS, nblk], [1, CLS]]),
                        in1=_sub(b2s[:], 0, [[0, nblk], [1, CLS]]),
                        op=mybir.AluOpType.add)
                    mx = pcb.tile([P, 4], f32, tag="mx")
                    nc.vector.tensor_reduce(
                        out=_sub(mx[:], 0, [[1, nblk]]),
                        in_=_sub(vv[:], 0, [[CLS, nblk], [1, CLS]]),
                        axis=mybir.AxisListType.X,
                        op=mybir.AluOpType.max)
                    u = pcb.tile([P, 4 * CLS], f32, tag="u")
                    nc.vector.tensor_tensor(
                        out=_sub(u[:], 0, [[CLS, nblk], [1, CLS]]),
                        in0=_sub(vv[:], 0, [[CLS, nblk], [1, CLS]]),
                        in1=_sub(mx[:], 0, [[1, nblk], [0, CLS]]),
                        op=mybir.AluOpType.subtract)
                    nc.scalar.activation(out=u[:, :nblk * CLS],
                                         in_=u[:, :nblk * CLS],
                                         func=mybir.ActivationFunctionType.Exp)
                    sm = pcb.tile([P, 4], f32, tag="sm")
                    nc.vector.tensor_reduce(
                        out=_sub(sm[:], 0, [[1, nblk]]),
                        in_=_sub(u[:], 0, [[CLS, nblk], [1, CLS]]),
                        axis=mybir.AxisListType.X,
                        op=mybir.AluOpType.add)
                    ls = pcb.tile([P, 4], f32, tag="ls")
                    nc.scalar.activation(out=ls[:, :nblk], in_=sm[:, :nblk],
                                         func=mybir.ActivationFunctionType.Ln)
                    nc.vector.tensor_tensor(out=ls[:, :nblk], in0=ls[:, :nblk],
                                            in1=mx[:, :nblk],
                                            op=mybir.AluOpType.add)
                    res = pcb.tile([P, 4 * CLS], f32, tag="res")
                    nc.vector.tensor_tensor(
                        out=_sub(res[:], 0, [[CLS, nblk], [1, CLS]]),
                        in0=_sub(vv[:], 0, [[CLS, nblk], [1, CLS]]),
                        in1=_sub(ls[:], 0, [[1, nblk], [0, CLS]]),
                        op=mybir.AluOpType.subtract)
                    nfull = sum(1 for b in sb["blocks"] if (b + 1) * P <= NPC)
                    if nfull:
                        nc.sync.dma_start(
                            out=bass.AP(out_d, b0 * P * CLS,
                                        [[CLS, P], [P * CLS, nfull], [1, CLS]]),
                            in_=_sub(res[:], 0, [[CLS, nfull], [1, CLS]]))
                    for bi, b in enumerate(sb["blocks"]):
                        if bi < nfull:
                            continue
                        rows = NPC - b * P
                        if rows > 0:
                            nc.sync.dma_start(
                                out=out_d[b * P:b * P + rows, :],
                                in_=res[:rows, bi * CLS:(bi + 1) * CLS])
            _pa_stack.close()
    nc.finalize()
    return nc


def install_ntff_hook(so_path="/opt/axon/libaxon_pjrt.so"):
    import types
    import ctypes
    import contextlib
    import antenv

    if getattr(antenv, "axon_hooks", None) is not None:
        return
    lib = ctypes.CDLL(so_path)
    if not hasattr(lib, "axon_start_nrt_profile"):
        return
    lib.axon_start_nrt_profile.argtypes = [ctypes.POINTER(ctypes.c_int64),
                                           ctypes.c_size_t]
    lib.axon_start_nrt_profile.restype = ctypes.c_int64
    lib.axon_stop_nrt_profile.argtypes = [ctypes.c_char_p]
    lib.axon_stop_nrt_profile.restype = ctypes.c_int64

    @contextlib.contextmanager
    def _hook(output_dir, device_ids):
        import jax
        jax.devices()
        if device_ids:
            ids = (ctypes.c_int64 * len(device_ids))(*device_ids)
            rc = lib.axon_start_nrt_profile(ids, len(device_ids))
        else:
            rc = lib.axon_start_nrt_profile(None, 0)
        if rc != 0:
            raise RuntimeError(f"axon_start_nrt_profile rc={rc}")
        try:
            yield
        finally:
            n = lib.axon_stop_nrt_profile(str(output_dir).encode())
            print(f"ntff profile: {n} file(s) written to {output_dir}")

    mod = types.ModuleType("antenv.axon_hooks")
    _reg = [_hook]
    mod.set_axon_ntff_profile_hook = lambda h: _reg.__setitem__(0, h)
    mod.get_axon_ntff_profile_hook = lambda: _reg[0]
    sys.modules["antenv.axon_hooks"] = mod
    antenv.axon_hooks = mod


def run(inputs, cfg, trace=False, **kwargs):
    if trace:
        install_ntff_hook()
    in_maps, meta = prep(inputs, cfg)
    nc = build(meta)
    res = bass_utils.run_bass_kernel_spmd(
        nc, in_maps, core_ids=list(range(cfg["NC"])), trace=trace, **kwargs)
    out = np.concatenate([res.results[c]["out"] for c in range(cfg["NC"])], axis=0)
    return out, res


# ----------------------------------------------------------------------------
# harness entry point
# ----------------------------------------------------------------------------

_CFG = dict(N=100000, F=165, H=4, C=64, CLS=2, NC=8)


def kernel(**inputs):
    """Full (unsharded) inputs -> full [N, 2] float32 log-softmax output.

    Shards edges by destination-node range across the 8 NeuronCores,
    compiles and runs the Bass/Tile kernel via run_bass_kernel_spmd,
    and concatenates the per-core output slices.
    """
    out, _ = run(inputs, _CFG, trace=False)
    return np.ascontiguousarray(out.astype(np.float32))
